# revision 49
# baseline (speedup 1.0000x reference)
"""Trainium2 Bass kernel for nn_DGEBlock (dense transformer block with
MoE-gated linears), distributed over 8 NeuronCores.

Sharding: data-parallel over batch (2 groups of 4 cores) x sequence-parallel
over tokens within each batch (512 tokens per core). Weights replicated.
Activations live feature-major ("T-layout": [d, tok]) in SBUF; V is
projected token-major (N-layout) so attention PV needs no transposes.

Precision plan (tolerance 2e-2): fp8-e4m3 DoubleRow matmuls (2 k-tiles per
instruction) for all six sigmoid-gate projections, the q/k/v/o mains and
attention PV/Z; plain fp8 for QK; bf16 for the two MLP mains (their error
passes unsquashed into the residual). Weights are pre-scaled x64 on host so
fp8 quantization stays in the normal range; epilogue scales fold the 64s
back out. GELU is computed via Erf (same ACT table set as Sigmoid, so the
MLP phase does zero activation-table reloads); LN uses Rsqrt directly.
K/V are gathered as fp8 (half the collective bytes), K first so its gather
hides under the V+Q projections.
"""

import sys

for _p in ("/opt/trn_rl_repo",):
    if _p not in sys.path:
        sys.path.append(_p)

import numpy as np
import ml_dtypes

# ---------------------------------------------------------------- constants
B = 2
T = 2048
D = 2048
H = 16
HD = 128
FF = 4 * D  # 8192
EPS = 1e-5

N_CORES = 8
GROUP = 4  # cores per batch group (sequence-parallel degree)
S = T // GROUP  # tokens per core = 512
P = 128
NT = D // P  # 16 feature tiles
NF = FF // P  # 64 hidden tiles
NKB = T // P  # 16 key blocks per batch
ISCALE = 1.0 / float(np.sqrt(HD))

WS = 64.0  # fp8 weight pre-scale (gate paths)
SA = 32.0  # q/k/v main-path weight pre-scale: keeps the fp8 activations
           # (32*k etc., |.| <~ 160) inside TRN e4m3's +-240 range
RSQRT2 = 1.0 / float(np.sqrt(2.0))

RG = [[0, 1, 2, 3], [4, 5, 6, 7]]

_BF = ml_dtypes.bfloat16
_F8 = ml_dtypes.float8_e4m3  # IEEE e4m3, max 240 == TRN FP8_EXP4

_COMPILED = None


# ------------------------------------------------------------- host prep
def _w_tiled_bf(W, scale=1.0):
    """W [dout, din] -> [nj, 128, nt, 128] bf16: W^T tile (t, j) layout;
    [j, p, t, jc] == W[j*128+jc, t*128+p]."""
    dout, din = W.shape
    nj, nt = dout // P, din // P
    return np.ascontiguousarray(
        (W.reshape(nj, P, nt, P) * scale).transpose(0, 3, 2, 1).astype(_BF)
    )


def _w_tiled_f8(W, scale=WS):
    """W [dout, din] -> [nj, 128, nt//2, 2, 128] fp8 DoubleRow pairs:
    [j, p, c, i, jc] == scale * W[j*128+jc, (2c+i)*128+p]."""
    dout, din = W.shape
    nj, nt = dout // P, din // P
    w = (W.reshape(nj, P, nt // 2, 2, P) * scale).transpose(0, 4, 2, 3, 1)
    w = np.clip(w, -240.0, 240.0)
    return np.ascontiguousarray(w.astype(_F8))


def _wT_pairs_f8(W, scale=WS):
    """W [dout, din] -> W^T pair layout [din//256, 128, 2, dout] fp8:
    [c, p, i, col] == scale * W[col, (2c+i)*128+p] (N-layout rhs)."""
    dout, din = W.shape
    w = (W.T.reshape(din // 256, 2, P, dout) * scale).transpose(0, 2, 1, 3)
    w = np.clip(w, -240.0, 240.0)
    return np.ascontiguousarray(w.astype(_F8))


def _b_cols(b, scale=1.0):
    """b [dout] -> [128, nj] fp32: column j holds b[j*128:(j+1)*128]."""
    nj = b.shape[0] // P
    return np.ascontiguousarray((b.reshape(nj, P) * scale).T.astype(np.float32))


# ------------------------------------------------------------- device build
def _build():
    from concourse import bacc, tile, mybir

    fp32 = mybir.dt.float32
    bf16 = mybir.dt.bfloat16
    fp8 = mybir.dt.float8e4
    AF = mybir.ActivationFunctionType
    ALU = mybir.AluOpType
    DR = mybir.MatmulPerfMode.DoubleRow

    nc = bacc.Bacc("TRN2", target_bir_lowering=False, debug=False,
                   num_devices=N_CORES)

    # ---- I/O tensors
    xT_d = nc.dram_tensor("xT", [D, S], bf16, kind="ExternalInput")
    wd = {}
    # fp8 DoubleRow pair weights (scale x64): qkvo mains + all gates
    for nm in ("Wq", "Wgq", "Wk", "Wgk", "Wo", "Wgo"):
        wd[nm] = nc.dram_tensor(nm, [NT, P, NT // 2, 2, P], fp8,
                                kind="ExternalInput")
    wd["Wgin"] = nc.dram_tensor("Wgin", [NF, P, NT // 2, 2, P], fp8,
                                kind="ExternalInput")
    wd["Wgout"] = nc.dram_tensor("Wgout", [NT, P, NF // 2, 2, P], fp8,
                                 kind="ExternalInput")
    # bf16 mains for the MLP
    wd["Win"] = nc.dram_tensor("Win", [NF, P, NT, P], bf16,
                               kind="ExternalInput")
    wd["Wout"] = nc.dram_tensor("Wout", [NT, P, NF, P], bf16,
                                kind="ExternalInput")
    # V projection runs in N-layout: W^T fp8 pair layout + bias rows (x64)
    wd["WvT"] = nc.dram_tensor("WvT", [NT // 2, P, 2, D], fp8,
                               kind="ExternalInput")
    wd["WgvT"] = nc.dram_tensor("WgvT", [NT // 2, P, 2, D], fp8,
                                kind="ExternalInput")
    bvrow_d = nc.dram_tensor("bvrow", [1, D], bf16, kind="ExternalInput")
    bgvrow_d = nc.dram_tensor("bgvrow", [1, D], bf16, kind="ExternalInput")
    bd = {}
    for nm in ("bq", "bgq", "bk", "bgk", "bo", "bgo",
               "bout", "bgout", "g1", "bt1", "g2", "bt2"):
        bd[nm] = nc.dram_tensor(nm, [P, NT], fp32, kind="ExternalInput")
    for nm in ("bin", "bgin"):
        bd[nm] = nc.dram_tensor(nm, [P, NF], fp32, kind="ExternalInput")
    out_d = nc.dram_tensor("outT", [D, S], fp32, kind="ExternalOutput")

    with tile.TileContext(nc) as tc:
        with (
            tc.tile_pool(name="const", bufs=1) as constp,
            tc.tile_pool(name="bias", bufs=1) as biasp,
            tc.tile_pool(name="rows", bufs=1) as rows,
            tc.tile_pool(name="dram", bufs=1, space="DRAM") as dramp,
        ):
            ones_col = constp.tile([P, 1], bf16)
            nc.vector.memset(ones_col[:], 1.0)
            ones_row = constp.tile([1, P], bf16)
            nc.vector.memset(ones_row[:], 1.0)
            # fp8 ones for DoubleRow Z-sums; padded so the pair-dim byte
            # stride is 16 (DoubleRow AP constraint)
            ones_z = constp.tile([P, 2, 16], fp8)
            nc.vector.memset(ones_z[:], 1.0)
            eps_t = constp.tile([1, 1], fp32)
            nc.vector.memset(eps_t[:], EPS)
            # bias/row constants load via the scalar queue so the sync
            # queue starts streaming x immediately
            bvrow = constp.tile([1, D], bf16)
            nc.scalar.dma_start(bvrow[:], bvrow_d.ap())
            bgvrow = constp.tile([1, D], bf16)
            nc.scalar.dma_start(bgvrow[:], bgvrow_d.ap())

            bias = {}
            for nm in bd:
                ncols = NF if nm in ("bin", "bgin") else NT
                btile = biasp.tile([P, ncols], fp32, name=f"bias_{nm}")
                nc.scalar.dma_start(btile[:], bd[nm].ap())
                bias[nm] = btile

            # ---------- helpers ----------
            def ln_stats(src_j, j, S1, S2, sqpool, name):
                """One tile's contribution to LN stats (callable per-j from
                an earlier phase's epilogue to hide the latency)."""
                sq = sqpool.tile([P, S], bf16, name=f"{name}_sq{j}",
                                 tag=f"{name}_sq", bufs=3)
                nc.scalar.activation(sq[:], src_j, AF.Square)
                nc.tensor.matmul(S1[:], ones_col[:], src_j,
                                 start=(j == 0), stop=(j == NT - 1))
                nc.tensor.matmul(S2[:], ones_col[:], sq[:],
                                 start=(j == 0), stop=(j == NT - 1))

            def ln_T(src, gname, bname, hpool, tmpool, psln, name,
                     stats=None):
                """LayerNorm over features of a T-layout activation.

                src: SBUF [128, NT, S] bf16 -> [128, NT, S] (dtype per
                hpool_dtype). Stats via ones-matmuls (or passed in via
                `stats` if accumulated earlier); per-token scale/shift rows
                broadcast via rank-1 matmuls; bf16 DVE affine (2x rate).
                """
                if stats is None:
                    S1 = psln.tile([1, S], fp32, name=f"{name}_S1",
                                   tag="ln_S1")
                    S2 = psln.tile([1, S], fp32, name=f"{name}_S2",
                                   tag="ln_S2")
                    for t in range(NT):
                        ln_stats(src[:, t, :], t, S1, S2, tmpool, name)
                else:
                    S1, S2 = stats

                def row(nm, dt=fp32):
                    return rows.tile([1, S], dt, name=f"{name}_{nm}",
                                     tag=f"ln_{nm}")

                mean = row("mean")
                nc.vector.tensor_scalar_mul(mean[:], S1[:], 1.0 / D)
                m2 = row("m2")
                nc.vector.tensor_scalar_mul(m2[:], S2[:], 1.0 / D)
                msq = row("msq")
                nc.vector.tensor_tensor(msq[:], mean[:], mean[:],
                                        op=ALU.mult)
                var = row("var")
                nc.vector.tensor_tensor(var[:], m2[:], msq[:],
                                        op=ALU.subtract)
                std = row("std")
                nc.scalar.activation(std[:], var[:], AF.Sqrt,
                                     bias=eps_t[:])
                rstd = row("rstd")
                nc.vector.reciprocal_approx_fast(rstd[:], std[:])
                rstd_bf = row("rstdbf", bf16)
                nc.vector.tensor_copy(rstd_bf[:], rstd[:])
                mr_bf = row("mrbf", bf16)
                nc.vector.tensor_tensor(mr_bf[:], mean[:], rstd[:],
                                        op=ALU.mult)
                Ab_p = psln.tile([P, S], fp32, name=f"{name}_Abp",
                                 tag="ln_Abp")
                nc.tensor.matmul(Ab_p[:], ones_row[:], rstd_bf[:])
                Bb_p = psln.tile([P, S], fp32, name=f"{name}_Bbp",
                                 tag="ln_Bbp")
                nc.tensor.matmul(Bb_p[:], ones_row[:], mr_bf[:])
                Ab = tmpool.tile([P, S], bf16, name=f"{name}_Ab")
                nc.vector.tensor_copy(Ab[:], Ab_p[:])
                Bb = tmpool.tile([P, S], bf16, name=f"{name}_Bb")
                nc.vector.tensor_copy(Bb[:], Bb_p[:])
                h = hpool.tile([P, NT, S], hpool_dtype[name],
                               name=f"{name}_h")
                for t in range(NT):
                    tmp = tmpool.tile([P, S], bf16, name=f"{name}_t0_{t}",
                                      tag="ln_t0", bufs=6)
                    nc.vector.tensor_tensor(tmp[:], src[:, t, :], Ab[:],
                                            op=ALU.mult)
                    tmp2 = tmpool.tile([P, S], bf16, name=f"{name}_t1_{t}",
                                       tag="ln_t1", bufs=6)
                    nc.vector.tensor_tensor(tmp2[:], tmp[:], Bb[:],
                                            op=ALU.subtract)
                    nc.scalar.activation(h[:, t, :], tmp2[:], AF.Identity,
                                         bias=bias[bname][:, t:t + 1],
                                         scale=bias[gname][:, t:t + 1])
                return h

            hpool_dtype = {"ln1": fp8, "ln2": bf16}

            def proj_gated_f8(src, nt, nj, wname, wgname, bgname, sig_scale,
                              wpool, pspool, epilogue, cchunk=None, wbufs=3):
                """Gated projection, T-layout, fp8 DoubleRow: for each output
                tile j, main/gate = sum_c W8(c,j).T @ src[:, 2c:2c+2, :],
                then epilogue(j, main_psum, sig_sbuf). src fp8 [P, nt, S]."""
                npair = nt // 2
                if cchunk is None:
                    cchunk = npair
                nchunk = npair // cchunk
                for j in range(nj):
                    main = pspool.tile([P, S], fp32, name=f"{wname}_m{j}",
                                       tag="pj_main", bufs=2)
                    gate = pspool.tile([P, S], fp32, name=f"{wname}_g{j}",
                                       tag="pj_gate", bufs=2)
                    for ci in range(nchunk):
                        wt = wpool.tile([P, cchunk, 2, P], fp8, tag="wmain",
                                        name=f"w_{wname}_{j}_{ci}",
                                        bufs=wbufs)
                        nc.sync.dma_start(
                            wt[:],
                            wd[wname].ap()[j, :,
                                           ci * cchunk:(ci + 1) * cchunk,
                                           :, :])
                        for cc in range(cchunk):
                            c = ci * cchunk + cc
                            nc.tensor.matmul(main[:], wt[:, cc, :, :],
                                             src[:, 2 * c:2 * c + 2, :],
                                             start=(c == 0),
                                             stop=(c == npair - 1),
                                             perf_mode=DR)
                    for ci in range(nchunk):
                        wg = wpool.tile([P, cchunk, 2, P], fp8, tag="wgate",
                                        name=f"w_{wgname}_{j}_{ci}",
                                        bufs=wbufs)
                        nc.sync.dma_start(
                            wg[:],
                            wd[wgname].ap()[j, :,
                                            ci * cchunk:(ci + 1) * cchunk,
                                            :, :])
                        for cc in range(cchunk):
                            c = ci * cchunk + cc
                            nc.tensor.matmul(gate[:], wg[:, cc, :, :],
                                             src[:, 2 * c:2 * c + 2, :],
                                             start=(c == 0),
                                             stop=(c == npair - 1),
                                             perf_mode=DR)
                    sig = wpool.tile([P, S], bf16, tag="sig",
                                     name=f"sig_{wname}_{j}", bufs=3)
                    nc.scalar.activation(sig[:], gate[:], AF.Sigmoid,
                                         bias=bias[bgname][:, j:j + 1],
                                         scale=sig_scale)
                    epilogue(j, main, sig)

            # x2 outlives phases A-C (used by LN2 + MLP residual)
            with tc.tile_pool(name="x2p", bufs=1) as x2p:
              with tc.tile_pool(name="xt", bufs=1) as xtp:
                xt = xtp.tile([P, NT, S], bf16)
                xT_v = xT_d.ap().rearrange("(t p) s -> t p s", p=P)
                for t in range(NT):
                    # split across two queues so two DMA rings fetch x in
                    # parallel (the gpsimd queue is idle this early)
                    eng = nc.sync if t % 2 == 0 else nc.gpsimd
                    eng.dma_start(xt[:, t, :], xT_v[t])

                vN_bounce = dramp.tile([S, D], fp8)
                k_bounce = dramp.tile([D, S], fp8)
                vgN = dramp.tile([GROUP * S, D], fp8)
                kg = dramp.tile([GROUP * D, S], fp8)

                with tc.tile_pool(name="yp", bufs=1) as ypool:
                  with tc.tile_pool(name="qp", bufs=1) as qpool:
                    q8 = qpool.tile([P, NT, S], fp8)

                    with tc.tile_pool(name="hq", bufs=1) as hqp:
                        with (
                            tc.tile_pool(name="ln1tmp", bufs=1) as ln1tmp,
                            tc.tile_pool(name="ln1ps", bufs=1,
                                         space="PSUM") as ln1ps,
                        ):
                            h1 = ln_T(xt, "g1", "bt1", hqp, ln1tmp, ln1ps,
                                      "ln1")

                        # Q weights are fully preloaded on the (otherwise
                        # idle) GpSimd DMA queue, spread through the K and V
                        # phases. The collectives HOL-block every DMA queued
                        # behind them, so Q must not need DMA after the
                        # gathers trigger.
                        # prefetch only the first half of Q's weights: the
                        # later j's compute after the gather window closes,
                        # so they can stream just-in-time on the sync queue
                        NQPRE = NT // 2
                        qw, qgw = {}, {}
                        for j in range(NQPRE):
                            qw[j] = hqp.tile([P, NT // 2, 2, P], fp8,
                                             tag="wqpre", bufs=NQPRE,
                                             name=f"wq_{j}")
                            qgw[j] = hqp.tile([P, NT // 2, 2, P], fp8,
                                              tag="wgqpre", bufs=NQPRE,
                                              name=f"wgq_{j}")

                        # V weight tiles: prefetched on the gpsimd queue
                        # during the K phase (V compute consumes them first;
                        # Q's prefetch follows during the V phase)
                        NP = NT // 2  # 8 contraction pairs
                        vw, vgw = {}, {}
                        for n in range(4):
                            for c in range(NP):
                                vw[n, c] = hqp.tile(
                                    [P, 2, S], fp8, tag="wv", bufs=32,
                                    name=f"wv_{n}_{c}")
                                vgw[n, c] = hqp.tile(
                                    [P, 2, S], fp8, tag="wgv", bufs=32,
                                    name=f"wgv_{n}_{c}")

                        def vw_prefetch_step(i):
                            n, c = divmod(i // 2, NP)
                            t = vgw[n, c] if i % 2 else vw[n, c]
                            w = wd["WgvT"] if i % 2 else wd["WvT"]
                            nc.gpsimd.dma_start(
                                t[:], w.ap()[c, :, :, n * S:(n + 1) * S])

                        # ---- K projection (T-layout); its AllGather
                        # triggers as soon as k_bounce is complete, which is
                        # why V/Q weights must already be in flight/SBUF ----
                        with (
                            tc.tile_pool(name="wproj", bufs=1) as wpool,
                            tc.tile_pool(name="pjps", bufs=1,
                                         space="PSUM") as pjps,
                        ):
                            def k_epi(j, main, sig):
                                kv = wpool.tile([P, S], fp8, tag="kv_out",
                                                name=f"kv_k_{j}", bufs=8)
                                nc.vector.scalar_tensor_tensor(
                                    kv[:], main[:], bias["bk"][:, j:j + 1],
                                    sig[:], op0=ALU.add, op1=ALU.mult)
                                nc.scalar.dma_start(
                                    k_bounce[j * P:(j + 1) * P, :], kv[:])
                                for i in range(4 * j, 4 * j + 4):
                                    vw_prefetch_step(i)

                            proj_gated_f8(h1, NT, NT, "Wk", "Wgk", "bgk",
                                          1.0 / WS, wpool, pjps, k_epi)

                        # ---- V projection, N-layout, fp8 DoubleRow.
                        # All V (and Q) weights are fully prefetched on the
                        # gpsimd queue before the K-gather can trigger: once
                        # a collective's DMA descriptors are on the rings,
                        # every later-queued DMA waits for peer delivery
                        # (~30-55us), so V/Q must not need loads then. ----
                        with (
                            tc.tile_pool(name="wv", bufs=1) as wvp,
                            tc.tile_pool(name="vps", bufs=1,
                                         space="PSUM") as vps,
                        ):
                            # Q weights (first half) prefetch during V
                            for j in range(NQPRE):
                                nc.gpsimd.dma_start(qw[j][:],
                                                    wd["Wq"].ap()[j])
                                nc.gpsimd.dma_start(qgw[j][:],
                                                    wd["Wgq"].ap()[j])
                            for n in range(4):
                                vmain = [vps.tile([P, S], fp32,
                                                  tag="v_main", bufs=4,
                                                  name=f"vm_{n}_{m}")
                                         for m in range(4)]
                                vgate = [vps.tile([P, S], fp32,
                                                  tag="v_gate", bufs=4,
                                                  name=f"vg_{n}_{m}")
                                         for m in range(4)]
                                for c in range(NP):
                                    wvt = vw[n, c]
                                    wgvt = vgw[n, c]
                                    for m in range(4):
                                        nc.tensor.matmul(
                                            vmain[m][:],
                                            h1[:, 2 * c:2 * c + 2,
                                               m * P:(m + 1) * P],
                                            wvt[:],
                                            start=(c == 0), stop=False,
                                            perf_mode=DR)
                                    for m in range(4):
                                        nc.tensor.matmul(
                                            vgate[m][:],
                                            h1[:, 2 * c:2 * c + 2,
                                               m * P:(m + 1) * P],
                                            wgvt[:],
                                            start=(c == 0), stop=False,
                                            perf_mode=DR)
                                for m in range(4):
                                    nc.tensor.matmul(
                                        vmain[m][:], ones_row[:],
                                        bvrow[:, n * S:(n + 1) * S],
                                        start=False, stop=True,
                                        skip_group_check=True)
                                    nc.tensor.matmul(
                                        vgate[m][:], ones_row[:],
                                        bgvrow[:, n * S:(n + 1) * S],
                                        start=False, stop=True,
                                        skip_group_check=True)
                                    vsig = wvp.tile([P, S], bf16,
                                                    tag="vsig", bufs=3,
                                                    name=f"vsig_{n}_{m}")
                                    nc.scalar.activation(vsig[:],
                                                         vgate[m][:],
                                                         AF.Sigmoid,
                                                         scale=1.0 / WS)
                                    vout = wvp.tile([P, S], fp8,
                                                    tag="vout", bufs=16,
                                                    name=f"vout_{n}_{m}")
                                    nc.vector.scalar_tensor_tensor(
                                        vout[:], vmain[m][:], 1.0, vsig[:],
                                        op0=ALU.mult, op1=ALU.mult)
                                    nc.scalar.dma_start(
                                        vN_bounce[m * P:(m + 1) * P,
                                                  n * S:(n + 1) * S],
                                        vout[:])

                        nc.gpsimd.collective_compute(
                            "AllGather", ALU.bypass, ins=[k_bounce[:]],
                            outs=[kg[:]], replica_groups=RG)
                        nc.gpsimd.collective_compute(
                            "AllGather", ALU.bypass, ins=[vN_bounce[:]],
                            outs=[vgN[:]], replica_groups=RG)

                        # ---- Q projection (weights already in SBUF) ----
                        with (
                            tc.tile_pool(name="wprojq", bufs=1) as wpoolq,
                            tc.tile_pool(name="pjqps", bufs=1,
                                         space="PSUM") as pjqps,
                        ):
                            for j in range(NT):
                                main = pjqps.tile([P, S], fp32,
                                                  name=f"Wq_m{j}",
                                                  tag="pj_main", bufs=2)
                                gate = pjqps.tile([P, S], fp32,
                                                  name=f"Wq_g{j}",
                                                  tag="pj_gate", bufs=2)
                                if j < NQPRE:
                                    wt, wg = qw[j], qgw[j]
                                else:
                                    wt = wpoolq.tile([P, NT // 2, 2, P],
                                                     fp8, tag="wmain",
                                                     name=f"w_Wq_{j}",
                                                     bufs=3)
                                    nc.sync.dma_start(wt[:],
                                                      wd["Wq"].ap()[j])
                                    wg = wpoolq.tile([P, NT // 2, 2, P],
                                                     fp8, tag="wgate",
                                                     name=f"w_Wgq_{j}",
                                                     bufs=3)
                                    nc.sync.dma_start(wg[:],
                                                      wd["Wgq"].ap()[j])
                                for c in range(NT // 2):
                                    nc.tensor.matmul(
                                        main[:], wt[:, c, :, :],
                                        h1[:, 2 * c:2 * c + 2, :],
                                        start=(c == 0),
                                        stop=(c == NT // 2 - 1),
                                        perf_mode=DR)
                                for c in range(NT // 2):
                                    nc.tensor.matmul(
                                        gate[:], wg[:, c, :, :],
                                        h1[:, 2 * c:2 * c + 2, :],
                                        start=(c == 0),
                                        stop=(c == NT // 2 - 1),
                                        perf_mode=DR)
                                sig = wpoolq.tile([P, S], bf16, tag="sig",
                                                  name=f"sig_Wq_{j}",
                                                  bufs=3)
                                nc.scalar.activation(
                                    sig[:], gate[:], AF.Sigmoid,
                                    bias=bias["bgq"][:, j:j + 1],
                                    scale=1.0 / WS)
                                nc.vector.scalar_tensor_tensor(
                                    q8[:, j, :], main[:],
                                    bias["bq"][:, j:j + 1],
                                    sig[:], op0=ALU.add, op1=ALU.mult)

                    # ---- phase B: attention (all fp8) ----
                    with (
                        tc.tile_pool(name="vres", bufs=1) as vresp,
                        tc.tile_pool(name="kstream", bufs=2) as kpool,
                        tc.tile_pool(name="apool", bufs=4) as apool,
                        tc.tile_pool(name="atps", bufs=1,
                                     space="PSUM") as atps,
                    ):
                        y8 = ypool.tile([P, NT, S], fp8)
                        Vt = vresp.tile([P, NKB, D], fp8)
                        for kb in range(NKB):
                            nc.gpsimd.dma_start(
                                Vt[:, kb, :],
                                vgN[kb * P:(kb + 1) * P, :])

                        head_state = {}

                        def finalize_head(h, Zp_h, Yp_h):
                            urow = rows.tile([1, S], fp32, name=f"u_{h}",
                                             tag="urow", bufs=2)
                            nc.vector.reciprocal_approx_fast(urow[:],
                                                             Zp_h[:])
                            ubf = rows.tile([1, S], bf16, name=f"ubf_{h}",
                                            tag="ubf", bufs=2)
                            nc.vector.tensor_copy(ubf[:], urow[:])
                            Us = apool.tile([P, S], bf16, tag="Us",
                                            name=f"Us_{h}", bufs=2)
                            nc.gpsimd.partition_broadcast(Us[:], ubf[:])
                            nc.vector.scalar_tensor_tensor(
                                y8[:, h, :], Yp_h[:], 1.0, Us[:],
                                op0=ALU.mult, op1=ALU.mult)

                        for hh in range(H):
                            # Kh loads ride the sync queue: the gpsimd
                            # queue is blocked until the V AllGather
                            # finishes, but kg is ready much earlier
                            Kh = kpool.tile([P, NKB * P], fp8, tag="Kh",
                                            name=f"Kh_{hh}")
                            for s_ in range(GROUP):
                                nc.sync.dma_start(
                                    Kh[:, s_ * S:(s_ + 1) * S],
                                    kg[s_ * D + hh * P:
                                       s_ * D + (hh + 1) * P, :])
                            Zp = atps.tile([1, S], fp32, name=f"Z_{hh}",
                                           tag="Zp", bufs=2)
                            Yp = atps.tile([P, S], fp32, name=f"Y_{hh}",
                                           tag="Yp", bufs=2)
                            ats = {}

                            def do_pair(c, hh=hh, Kh=Kh, ats=ats):
                                Lp2 = atps.tile([P, 2, S], fp32,
                                                name=f"L_{hh}_{c}",
                                                tag="logits", bufs=2)
                                for i in range(2):
                                    kb = 2 * c + i
                                    nc.tensor.matmul(
                                        Lp2[:, i, :],
                                        Kh[:, kb * P:(kb + 1) * P],
                                        q8[:, hh, :])
                                At2 = apool.tile(
                                    [P, 2, S], fp8, tag="At",
                                    name=f"At_{hh}_{c}", bufs=4)
                                nc.scalar.activation(
                                    At2[:], Lp2[:],
                                    AF.Exp, scale=ISCALE / (SA * SA))
                                ats[c] = At2

                            do_pair(0)
                            do_pair(1)
                            for c in range(NKB // 2):
                                if c + 2 < NKB // 2:
                                    do_pair(c + 2)
                                At2 = ats[c]
                                nc.tensor.matmul(Zp[:],
                                                 ones_z[:, :, 0:1],
                                                 At2[:],
                                                 start=(c == 0),
                                                 stop=(c == NKB // 2 - 1),
                                                 perf_mode=DR)
                                nc.tensor.matmul(
                                    Yp[:],
                                    Vt[:, 2 * c:2 * c + 2,
                                       hh * P:(hh + 1) * P],
                                    At2[:],
                                    start=(c == 0),
                                    stop=(c == NKB // 2 - 1),
                                    perf_mode=DR)
                                if c == 2 and hh > 0:
                                    finalize_head(hh - 1,
                                                  *head_state[hh - 1])
                            head_state[hh] = (Zp, Yp)
                        finalize_head(H - 1, *head_state[H - 1])

                  # ---- phase C: o-proj + residual (fp8, y at scale 32);
                  # LN2 stats accumulate per-j right here so phase D's
                  # LayerNorm only has the row chain + affine left ----
                  x2 = x2p.tile([P, NT, S], bf16, name="x2")
                  ln2S1s = rows.tile([1, S], fp32, name="ln2_S1s")
                  ln2S2s = rows.tile([1, S], fp32, name="ln2_S2s")
                  with (
                      tc.tile_pool(name="wproj2", bufs=1) as wpool2,
                      tc.tile_pool(name="pj2ps", bufs=1,
                                   space="PSUM") as pj2ps,
                  ):
                      ln2S1 = pj2ps.tile([1, S], fp32, name="ln2_S1")
                      ln2S2 = pj2ps.tile([1, S], fp32, name="ln2_S2")
                      def o_epi(j, main, sig):
                          tmp = wpool2.tile([P, S], fp32, tag="o_tmp",
                                            name=f"o_tmp_{j}", bufs=3)
                          nc.vector.scalar_tensor_tensor(
                              tmp[:], main[:], bias["bo"][:, j:j + 1],
                              sig[:], op0=ALU.add, op1=ALU.mult)
                          nc.vector.scalar_tensor_tensor(
                              x2[:, j, :], tmp[:], 1.0 / (SA * WS),
                              xt[:, j, :], op0=ALU.mult, op1=ALU.add)
                          ln_stats(x2[:, j, :], j, ln2S1, ln2S2, wpool2,
                                   "ln2")

                      proj_gated_f8(y8, NT, NT, "Wo", "Wgo", "bgo",
                                    1.0 / (SA * WS), wpool2, pj2ps, o_epi)
                      nc.vector.tensor_copy(ln2S1s[:], ln2S1[:])
                      nc.vector.tensor_copy(ln2S2s[:], ln2S2[:])

              # ---- phase D: LN2 + MLP (mains bf16, gates fp8); one psum
              # pool spans both MLP stages so the ring never drains ----
              with (
                  tc.tile_pool(name="midp", bufs=1) as midp,
                  tc.tile_pool(name="mlpps", bufs=1, space="PSUM") as mlpps,
              ):
                  mid = midp.tile([P, NF, S], bf16)
                  mid8 = midp.tile([P, NF, S], fp8)
                  with tc.tile_pool(name="h2p", bufs=1) as h2p:
                      with (
                          tc.tile_pool(name="ln2tmp", bufs=1) as ln2tmp,
                          tc.tile_pool(name="ln2ps", bufs=1,
                                       space="PSUM") as ln2ps,
                      ):
                          h2 = ln_T(x2, "g2", "bt2", h2p, ln2tmp, ln2ps,
                                    "ln2", stats=(ln2S1s, ln2S2s))
                      h2q = h2p.tile([P, NT, S], fp8, name="h2q")
                      for t in range(NT):
                          nc.scalar.activation(h2q[:, t, :], h2[:, t, :],
                                               AF.Identity)

                      with tc.tile_pool(name="wmlp1", bufs=1) as wm1:
                          # main (bf16) accumulation + fp8 gate; epilogue:
                          # t1 = (z + b) * sig; e = erf(t1 / sqrt2);
                          # mid = (e + 1) * t1 = 2*gelu (0.5 folded into
                          # W_out/Wg_out host-side); mid8 = fp8 copy.
                          for j in range(NF):
                              main = mlpps.tile([P, S], fp32,
                                               name=f"Win_m{j}",
                                               tag="pj_main", bufs=2)
                              gate = mlpps.tile([P, S], fp32,
                                               name=f"Win_g{j}",
                                               tag="pj_gate", bufs=2)
                              wt = wm1.tile([P, NT, P], bf16, tag="wmain",
                                            name=f"w_Win_{j}", bufs=3)
                              nc.sync.dma_start(wt[:], wd["Win"].ap()[j])
                              for t in range(NT):
                                  nc.tensor.matmul(main[:], wt[:, t, :],
                                                   h2[:, t, :],
                                                   start=(t == 0),
                                                   stop=(t == NT - 1))
                              wg = wm1.tile([P, NT // 2, 2, P], fp8,
                                            tag="wgate",
                                            name=f"w_Wgin_{j}", bufs=3)
                              nc.sync.dma_start(wg[:], wd["Wgin"].ap()[j])
                              for c in range(NT // 2):
                                  nc.tensor.matmul(
                                      gate[:], wg[:, c, :, :],
                                      h2q[:, 2 * c:2 * c + 2, :],
                                      start=(c == 0),
                                      stop=(c == NT // 2 - 1),
                                      perf_mode=DR)
                              sig = wm1.tile([P, S], bf16, tag="sig",
                                             name=f"sig_Win_{j}", bufs=3)
                              nc.scalar.activation(
                                  sig[:], gate[:], AF.Sigmoid,
                                  bias=bias["bgin"][:, j:j + 1],
                                  scale=1.0 / WS)
                              t1 = wm1.tile([P, S], bf16, tag="mid_t1",
                                            name=f"t1_{j}", bufs=3)
                              nc.vector.scalar_tensor_tensor(
                                  t1[:], main[:],
                                  bias["bin"][:, j:j + 1], sig[:],
                                  op0=ALU.add, op1=ALU.mult)
                              er = wm1.tile([P, S], bf16, tag="mid_er",
                                            name=f"er_{j}", bufs=3)
                              nc.scalar.activation(er[:], t1[:], AF.Erf,
                                                   scale=RSQRT2)
                              nc.vector.scalar_tensor_tensor(
                                  mid[:, j, :], er[:], 1.0, t1[:],
                                  op0=ALU.add, op1=ALU.mult)
                              nc.scalar.activation(mid8[:, j, :],
                                                   mid[:, j, :],
                                                   AF.Identity)

                  with tc.tile_pool(name="wmlp2", bufs=1) as wm2:
                      # W_out main bf16 (x0.5 host), gate fp8 (x32 host)
                      for j in range(NT):
                          main = mlpps.tile([P, S], fp32,
                                           name=f"Wout_m{j}",
                                           tag="pj_main", bufs=2)
                          gate = mlpps.tile([P, S], fp32,
                                           name=f"Wout_g{j}",
                                           tag="pj_gate", bufs=2)
                          for ci in range(2):
                              wt = wm2.tile([P, 32, P], bf16, tag="wmain",
                                            name=f"w_Wout_{j}_{ci}",
                                            bufs=2)
                              nc.sync.dma_start(
                                  wt[:],
                                  wd["Wout"].ap()[j, :,
                                                  32 * ci:32 * (ci + 1),
                                                  :])
                              for tt_ in range(32):
                                  t = 32 * ci + tt_
                                  nc.tensor.matmul(main[:], wt[:, tt_, :],
                                                   mid[:, t, :],
                                                   start=(t == 0),
                                                   stop=(t == NF - 1))
                          for ci in range(2):
                              wg = wm2.tile([P, 16, 2, P], fp8,
                                            tag="wgate",
                                            name=f"w_Wgout_{j}_{ci}",
                                            bufs=2)
                              nc.sync.dma_start(
                                  wg[:],
                                  wd["Wgout"].ap()[j, :,
                                                   16 * ci:16 * (ci + 1),
                                                   :, :])
                              for cc in range(16):
                                  c = 16 * ci + cc
                                  nc.tensor.matmul(
                                      gate[:], wg[:, cc, :, :],
                                      mid8[:, 2 * c:2 * c + 2, :],
                                      start=(c == 0),
                                      stop=(c == NF // 2 - 1),
                                      perf_mode=DR)
                          sig = wm2.tile([P, S], bf16, tag="sig",
                                         name=f"sig_Wout_{j}", bufs=3)
                          nc.scalar.activation(
                              sig[:], gate[:], AF.Sigmoid,
                              bias=bias["bgout"][:, j:j + 1],
                              scale=1.0 / WS)
                          tmp = wm2.tile([P, S], fp32, tag="out_tmp",
                                         name=f"out_tmp_{j}", bufs=3)
                          nc.vector.scalar_tensor_tensor(
                              tmp[:], main[:], bias["bout"][:, j:j + 1],
                              sig[:], op0=ALU.add, op1=ALU.mult)
                          outf = wm2.tile([P, S], fp32, tag="out_f",
                                          name=f"out_f_{j}", bufs=3)
                          nc.vector.tensor_tensor(outf[:], tmp[:],
                                                  x2[:, j, :], op=ALU.add)
                          nc.sync.dma_start(
                              out_d.ap()[j * P:(j + 1) * P, :], outf[:])

    nc.compile()
    return nc


def _prep_shared_inputs(inputs):
    m = {}
    for nm, w, sc in (("Wq", "W_q", SA), ("Wgq", "Wg_q", WS),
                      ("Wk", "W_k", SA), ("Wgk", "Wg_k", WS),
                      ("Wo", "W_o", WS), ("Wgo", "Wg_o", WS)):
        m[nm] = _w_tiled_f8(np.asarray(inputs[w]), sc)
    m["Wgin"] = _w_tiled_f8(np.asarray(inputs["Wg_in"]), WS)
    m["Wgout"] = _w_tiled_f8(np.asarray(inputs["Wg_out"]), WS / 2.0)
    m["Win"] = _w_tiled_bf(np.asarray(inputs["W_in"]), 1.0)
    m["Wout"] = _w_tiled_bf(np.asarray(inputs["W_out"]), 0.5)
    m["WvT"] = _wT_pairs_f8(np.asarray(inputs["W_v"]), SA)
    m["WgvT"] = _wT_pairs_f8(np.asarray(inputs["Wg_v"]), WS)
    m["bvrow"] = (np.asarray(inputs["b_v"]) * SA).astype(_BF).reshape(1, D)
    m["bgvrow"] = (np.asarray(inputs["bg_v"]) * WS).astype(_BF).reshape(1, D)
    for nm, bn, sc in (("bq", "b_q", SA), ("bgq", "bg_q", 1.0),
                       ("bk", "b_k", SA), ("bgk", "bg_k", 1.0),
                       ("bo", "b_o", SA * WS), ("bgo", "bg_o", 1.0),
                       ("bin", "b_in", 1.0), ("bgin", "bg_in", 1.0),
                       ("bout", "b_out", 1.0), ("bgout", "bg_out", 1.0),
                       ("g1", "ln1_g", 1.0), ("bt1", "ln1_b", 1.0),
                       ("g2", "ln2_g", 1.0), ("bt2", "ln2_b", 1.0)):
        m[nm] = _b_cols(np.asarray(inputs[bn]), sc)
    return m


def _install_trace_shim():
    """Provide antenv.axon_hooks (NTFF profiling) if the image lacks it."""
    import contextlib
    import ctypes
    import types

    try:
        import antenv.axon_hooks  # noqa: F401
        return
    except ImportError:
        pass
    try:
        import antenv
    except ImportError:
        return
    so_path = "/opt/axon/libaxon_pjrt.so"
    try:
        lib = ctypes.CDLL(so_path)
    except OSError:
        return
    if not hasattr(lib, "axon_start_nrt_profile"):
        return
    lib.axon_start_nrt_profile.argtypes = [ctypes.POINTER(ctypes.c_int64),
                                           ctypes.c_size_t]
    lib.axon_start_nrt_profile.restype = ctypes.c_int64
    lib.axon_stop_nrt_profile.argtypes = [ctypes.c_char_p]
    lib.axon_stop_nrt_profile.restype = ctypes.c_int64

    @contextlib.contextmanager
    def hook(output_dir, device_ids):
        import jax

        jax.devices()
        if device_ids:
            ids = (ctypes.c_int64 * len(device_ids))(*device_ids)
            rc = lib.axon_start_nrt_profile(ids, len(device_ids))
        else:
            rc = lib.axon_start_nrt_profile(None, 0)
        if rc != 0:
            raise RuntimeError(f"axon_start_nrt_profile rc={rc}")
        try:
            yield
        finally:
            n = lib.axon_stop_nrt_profile(str(output_dir).encode())
            print(f"profile: {n} ntff file(s) in {output_dir}",
                  file=sys.stderr)

    mod = types.ModuleType("antenv.axon_hooks")
    mod.get_axon_ntff_profile_hook = lambda: hook
    mod.set_axon_ntff_profile_hook = lambda h: None
    sys.modules["antenv.axon_hooks"] = mod
    antenv.axon_hooks = mod


LAST_RESULTS = None


def kernel(_trace=False, **inputs):
    global _COMPILED, LAST_RESULTS
    from concourse import bass_utils

    if _trace:
        _install_trace_shim()

    if _COMPILED is None:
        _COMPILED = _build()
    nc = _COMPILED

    shared = _prep_shared_inputs(inputs)
    x = np.asarray(inputs["x"], dtype=np.float32)  # [B, T, D]
    in_maps = []
    for c in range(N_CORES):
        g, s = divmod(c, GROUP)
        xT_c = np.ascontiguousarray(x[g, s * S:(s + 1) * S, :].T.astype(_BF))
        m = dict(shared)
        m["xT"] = xT_c
        in_maps.append(m)

    LAST_RESULTS = bass_utils.run_bass_kernel_spmd(
        nc, in_maps, core_ids=list(range(N_CORES)), trace=_trace)

    out = np.empty((B, T, D), dtype=np.float32)
    for c in range(N_CORES):
        g, s = divmod(c, GROUP)
        out[g, s * S:(s + 1) * S, :] = LAST_RESULTS.results[c]["outT"].T
    return out


# revision 50
# speedup vs baseline: 1.0029x; 1.0029x over previous
"""Trainium2 Bass kernel for nn_DGEBlock (dense transformer block with
MoE-gated linears), distributed over 8 NeuronCores.

Sharding: data-parallel over batch (2 groups of 4 cores) x sequence-parallel
over tokens within each batch (512 tokens per core). Weights replicated.
Activations live feature-major ("T-layout": [d, tok]) in SBUF; V is
projected token-major (N-layout) so attention PV needs no transposes.

Precision plan (tolerance 2e-2): fp8-e4m3 DoubleRow matmuls (2 k-tiles per
instruction, ~2x PE throughput) for all six sigmoid-gate projections, the
q/k/v/o mains and attention PV/Z; plain fp8 for QK; bf16 for the two MLP
mains (their error passes unsquashed into the residual; measured sim error
for this mix is 1.1e-2 vs 1.9e-2 with an MLP main in fp8). Weights are
pre-scaled on host (x64 gates, x32 qkv mains so no fp8 activation exceeds
TRN e4m3's +-240 -> Inf); epilogue scales fold everything back out. GELU is
computed via Erf (same ACT table set as Sigmoid -> no table thrash in the
MLP phase; the 0.5 folds into W_out host-side).

Scheduling: K/V are gathered as fp8. A collective's DMA descriptors
head-of-line block every DMA queued behind them until all peers deliver
(~30-55us of skew), so the V and first-half-Q weights are fully prefetched
into SBUF on the gpsimd queue before the K gather can trigger, and the
attention Kh loads ride the sync queue (the gpsimd queue is parked on the
V gather). LN2 statistics accumulate inside the o-projection epilogues;
paired logit-EXPs ([128,2,512] PSUM reads) halve ACT instruction count in
attention; softmax 1/Z runs via reciprocal_approx_fast + a gpsimd
partition_broadcast instead of a PE broadcast matmul.
"""

import sys

for _p in ("/opt/trn_rl_repo",):
    if _p not in sys.path:
        sys.path.append(_p)

import numpy as np
import ml_dtypes

# ---------------------------------------------------------------- constants
B = 2
T = 2048
D = 2048
H = 16
HD = 128
FF = 4 * D  # 8192
EPS = 1e-5

N_CORES = 8
GROUP = 4  # cores per batch group (sequence-parallel degree)
S = T // GROUP  # tokens per core = 512
P = 128
NT = D // P  # 16 feature tiles
NF = FF // P  # 64 hidden tiles
NKB = T // P  # 16 key blocks per batch
ISCALE = 1.0 / float(np.sqrt(HD))

WS = 64.0  # fp8 weight pre-scale (gate paths)
SA = 32.0  # q/k/v main-path weight pre-scale: keeps the fp8 activations
           # (32*k etc., |.| <~ 160) inside TRN e4m3's +-240 range
RSQRT2 = 1.0 / float(np.sqrt(2.0))

RG = [[0, 1, 2, 3], [4, 5, 6, 7]]

_BF = ml_dtypes.bfloat16
_F8 = ml_dtypes.float8_e4m3  # IEEE e4m3, max 240 == TRN FP8_EXP4

_COMPILED = None


# ------------------------------------------------------------- host prep
def _w_tiled_bf(W, scale=1.0):
    """W [dout, din] -> [nj, 128, nt, 128] bf16: W^T tile (t, j) layout;
    [j, p, t, jc] == W[j*128+jc, t*128+p]."""
    dout, din = W.shape
    nj, nt = dout // P, din // P
    return np.ascontiguousarray(
        (W.reshape(nj, P, nt, P) * scale).transpose(0, 3, 2, 1).astype(_BF)
    )


def _w_tiled_f8(W, scale=WS):
    """W [dout, din] -> [nj, 128, nt//2, 2, 128] fp8 DoubleRow pairs:
    [j, p, c, i, jc] == scale * W[j*128+jc, (2c+i)*128+p]."""
    dout, din = W.shape
    nj, nt = dout // P, din // P
    w = (W.reshape(nj, P, nt // 2, 2, P) * scale).transpose(0, 4, 2, 3, 1)
    w = np.clip(w, -240.0, 240.0)
    return np.ascontiguousarray(w.astype(_F8))


def _wT_pairs_f8(W, scale=WS):
    """W [dout, din] -> W^T pair layout [din//256, 128, 2, dout] fp8:
    [c, p, i, col] == scale * W[col, (2c+i)*128+p] (N-layout rhs)."""
    dout, din = W.shape
    w = (W.T.reshape(din // 256, 2, P, dout) * scale).transpose(0, 2, 1, 3)
    w = np.clip(w, -240.0, 240.0)
    return np.ascontiguousarray(w.astype(_F8))


def _b_cols(b, scale=1.0):
    """b [dout] -> [128, nj] fp32: column j holds b[j*128:(j+1)*128]."""
    nj = b.shape[0] // P
    return np.ascontiguousarray((b.reshape(nj, P) * scale).T.astype(np.float32))


# ------------------------------------------------------------- device build
def _build():
    from concourse import bacc, tile, mybir

    fp32 = mybir.dt.float32
    bf16 = mybir.dt.bfloat16
    fp8 = mybir.dt.float8e4
    AF = mybir.ActivationFunctionType
    ALU = mybir.AluOpType
    DR = mybir.MatmulPerfMode.DoubleRow

    nc = bacc.Bacc("TRN2", target_bir_lowering=False, debug=False,
                   num_devices=N_CORES)

    # ---- I/O tensors
    xT_d = nc.dram_tensor("xT", [D, S], bf16, kind="ExternalInput")
    wd = {}
    # fp8 DoubleRow pair weights (scale x64): qkvo mains + all gates
    for nm in ("Wq", "Wgq", "Wk", "Wgk", "Wo", "Wgo"):
        wd[nm] = nc.dram_tensor(nm, [NT, P, NT // 2, 2, P], fp8,
                                kind="ExternalInput")
    wd["Wgin"] = nc.dram_tensor("Wgin", [NF, P, NT // 2, 2, P], fp8,
                                kind="ExternalInput")
    wd["Wgout"] = nc.dram_tensor("Wgout", [NT, P, NF // 2, 2, P], fp8,
                                 kind="ExternalInput")
    # bf16 mains for the MLP
    wd["Win"] = nc.dram_tensor("Win", [NF, P, NT, P], bf16,
                               kind="ExternalInput")
    wd["Wout"] = nc.dram_tensor("Wout", [NT, P, NF, P], bf16,
                                kind="ExternalInput")
    # V projection runs in N-layout: W^T fp8 pair layout + bias rows (x64)
    wd["WvT"] = nc.dram_tensor("WvT", [NT // 2, P, 2, D], fp8,
                               kind="ExternalInput")
    wd["WgvT"] = nc.dram_tensor("WgvT", [NT // 2, P, 2, D], fp8,
                                kind="ExternalInput")
    bvrow_d = nc.dram_tensor("bvrow", [1, D], bf16, kind="ExternalInput")
    bgvrow_d = nc.dram_tensor("bgvrow", [1, D], bf16, kind="ExternalInput")
    bd = {}
    for nm in ("bq", "bgq", "bk", "bgk", "bo", "bgo",
               "bout", "bgout", "g1", "bt1", "g2", "bt2"):
        bd[nm] = nc.dram_tensor(nm, [P, NT], fp32, kind="ExternalInput")
    for nm in ("bin", "bgin"):
        bd[nm] = nc.dram_tensor(nm, [P, NF], fp32, kind="ExternalInput")
    out_d = nc.dram_tensor("outT", [D, S], fp32, kind="ExternalOutput")

    with tile.TileContext(nc) as tc:
        with (
            tc.tile_pool(name="const", bufs=1) as constp,
            tc.tile_pool(name="bias", bufs=1) as biasp,
            tc.tile_pool(name="rows", bufs=1) as rows,
            tc.tile_pool(name="dram", bufs=1, space="DRAM") as dramp,
        ):
            ones_col = constp.tile([P, 1], bf16)
            nc.vector.memset(ones_col[:], 1.0)
            ones_row = constp.tile([1, P], bf16)
            nc.vector.memset(ones_row[:], 1.0)
            # fp8 ones for DoubleRow Z-sums; padded so the pair-dim byte
            # stride is 16 (DoubleRow AP constraint)
            ones_z = constp.tile([P, 2, 16], fp8)
            nc.vector.memset(ones_z[:], 1.0)
            eps_t = constp.tile([1, 1], fp32)
            nc.vector.memset(eps_t[:], EPS)
            # bias/row constants load via the scalar queue so the sync
            # queue starts streaming x immediately
            bvrow = constp.tile([1, D], bf16)
            nc.scalar.dma_start(bvrow[:], bvrow_d.ap())
            bgvrow = constp.tile([1, D], bf16)
            nc.scalar.dma_start(bgvrow[:], bgvrow_d.ap())

            bias = {}
            for nm in bd:
                ncols = NF if nm in ("bin", "bgin") else NT
                btile = biasp.tile([P, ncols], fp32, name=f"bias_{nm}")
                nc.scalar.dma_start(btile[:], bd[nm].ap())
                bias[nm] = btile

            # ---------- helpers ----------
            def ln_stats(src_j, j, S1, S2, sqpool, name):
                """One tile's contribution to LN stats (callable per-j from
                an earlier phase's epilogue to hide the latency)."""
                sq = sqpool.tile([P, S], bf16, name=f"{name}_sq{j}",
                                 tag=f"{name}_sq", bufs=3)
                nc.scalar.activation(sq[:], src_j, AF.Square)
                nc.tensor.matmul(S1[:], ones_col[:], src_j,
                                 start=(j == 0), stop=(j == NT - 1))
                nc.tensor.matmul(S2[:], ones_col[:], sq[:],
                                 start=(j == 0), stop=(j == NT - 1))

            def ln_T(src, gname, bname, hpool, tmpool, psln, name,
                     stats=None):
                """LayerNorm over features of a T-layout activation.

                src: SBUF [128, NT, S] bf16 -> [128, NT, S] (dtype per
                hpool_dtype). Stats via ones-matmuls (or passed in via
                `stats` if accumulated earlier); per-token scale/shift rows
                broadcast via rank-1 matmuls; bf16 DVE affine (2x rate).
                """
                if stats is None:
                    S1 = psln.tile([1, S], fp32, name=f"{name}_S1",
                                   tag="ln_S1")
                    S2 = psln.tile([1, S], fp32, name=f"{name}_S2",
                                   tag="ln_S2")
                    for t in range(NT):
                        ln_stats(src[:, t, :], t, S1, S2, tmpool, name)
                else:
                    S1, S2 = stats

                def row(nm, dt=fp32):
                    return rows.tile([1, S], dt, name=f"{name}_{nm}",
                                     tag=f"ln_{nm}")

                mean = row("mean")
                nc.vector.tensor_scalar_mul(mean[:], S1[:], 1.0 / D)
                m2 = row("m2")
                nc.vector.tensor_scalar_mul(m2[:], S2[:], 1.0 / D)
                msq = row("msq")
                nc.vector.tensor_tensor(msq[:], mean[:], mean[:],
                                        op=ALU.mult)
                var = row("var")
                nc.vector.tensor_tensor(var[:], m2[:], msq[:],
                                        op=ALU.subtract)
                std = row("std")
                nc.scalar.activation(std[:], var[:], AF.Sqrt,
                                     bias=eps_t[:])
                rstd = row("rstd")
                nc.vector.reciprocal_approx_fast(rstd[:], std[:])
                rstd_bf = row("rstdbf", bf16)
                nc.vector.tensor_copy(rstd_bf[:], rstd[:])
                mr_bf = row("mrbf", bf16)
                nc.vector.tensor_tensor(mr_bf[:], mean[:], rstd[:],
                                        op=ALU.mult)
                Ab_p = psln.tile([P, S], fp32, name=f"{name}_Abp",
                                 tag="ln_Abp")
                nc.tensor.matmul(Ab_p[:], ones_row[:], rstd_bf[:])
                Bb_p = psln.tile([P, S], fp32, name=f"{name}_Bbp",
                                 tag="ln_Bbp")
                nc.tensor.matmul(Bb_p[:], ones_row[:], mr_bf[:])
                Ab = tmpool.tile([P, S], bf16, name=f"{name}_Ab")
                nc.vector.tensor_copy(Ab[:], Ab_p[:])
                Bb = tmpool.tile([P, S], bf16, name=f"{name}_Bb")
                nc.vector.tensor_copy(Bb[:], Bb_p[:])
                h = hpool.tile([P, NT, S], hpool_dtype[name],
                               name=f"{name}_h")
                for t in range(NT):
                    tmp = tmpool.tile([P, S], bf16, name=f"{name}_t0_{t}",
                                      tag="ln_t0", bufs=6)
                    nc.vector.tensor_tensor(tmp[:], src[:, t, :], Ab[:],
                                            op=ALU.mult)
                    tmp2 = tmpool.tile([P, S], bf16, name=f"{name}_t1_{t}",
                                       tag="ln_t1", bufs=6)
                    nc.vector.tensor_tensor(tmp2[:], tmp[:], Bb[:],
                                            op=ALU.subtract)
                    nc.scalar.activation(h[:, t, :], tmp2[:], AF.Identity,
                                         bias=bias[bname][:, t:t + 1],
                                         scale=bias[gname][:, t:t + 1])
                return h

            hpool_dtype = {"ln1": fp8, "ln2": bf16}

            def proj_gated_f8(src, nt, nj, wname, wgname, bgname, sig_scale,
                              wpool, pspool, epilogue, cchunk=None, wbufs=3):
                """Gated projection, T-layout, fp8 DoubleRow: for each output
                tile j, main/gate = sum_c W8(c,j).T @ src[:, 2c:2c+2, :],
                then epilogue(j, main_psum, sig_sbuf). src fp8 [P, nt, S]."""
                npair = nt // 2
                if cchunk is None:
                    cchunk = npair
                nchunk = npair // cchunk
                for j in range(nj):
                    main = pspool.tile([P, S], fp32, name=f"{wname}_m{j}",
                                       tag="pj_main", bufs=2)
                    gate = pspool.tile([P, S], fp32, name=f"{wname}_g{j}",
                                       tag="pj_gate", bufs=2)
                    for ci in range(nchunk):
                        wt = wpool.tile([P, cchunk, 2, P], fp8, tag="wmain",
                                        name=f"w_{wname}_{j}_{ci}",
                                        bufs=wbufs)
                        nc.sync.dma_start(
                            wt[:],
                            wd[wname].ap()[j, :,
                                           ci * cchunk:(ci + 1) * cchunk,
                                           :, :])
                        for cc in range(cchunk):
                            c = ci * cchunk + cc
                            nc.tensor.matmul(main[:], wt[:, cc, :, :],
                                             src[:, 2 * c:2 * c + 2, :],
                                             start=(c == 0),
                                             stop=(c == npair - 1),
                                             perf_mode=DR)
                    for ci in range(nchunk):
                        wg = wpool.tile([P, cchunk, 2, P], fp8, tag="wgate",
                                        name=f"w_{wgname}_{j}_{ci}",
                                        bufs=wbufs)
                        nc.sync.dma_start(
                            wg[:],
                            wd[wgname].ap()[j, :,
                                            ci * cchunk:(ci + 1) * cchunk,
                                            :, :])
                        for cc in range(cchunk):
                            c = ci * cchunk + cc
                            nc.tensor.matmul(gate[:], wg[:, cc, :, :],
                                             src[:, 2 * c:2 * c + 2, :],
                                             start=(c == 0),
                                             stop=(c == npair - 1),
                                             perf_mode=DR)
                    sig = wpool.tile([P, S], bf16, tag="sig",
                                     name=f"sig_{wname}_{j}", bufs=3)
                    nc.scalar.activation(sig[:], gate[:], AF.Sigmoid,
                                         bias=bias[bgname][:, j:j + 1],
                                         scale=sig_scale)
                    epilogue(j, main, sig)

            # x2 outlives phases A-C (used by LN2 + MLP residual)
            with tc.tile_pool(name="x2p", bufs=1) as x2p:
              with tc.tile_pool(name="xt", bufs=1) as xtp:
                xt = xtp.tile([P, NT, S], bf16)
                xT_v = xT_d.ap().rearrange("(t p) s -> t p s", p=P)
                for t in range(NT):
                    # split across two queues so two DMA rings fetch x in
                    # parallel (the gpsimd queue is idle this early)
                    eng = nc.sync if t % 2 == 0 else nc.gpsimd
                    eng.dma_start(xt[:, t, :], xT_v[t])

                vN_bounce = dramp.tile([S, D], fp8)
                k_bounce = dramp.tile([D, S], fp8)
                vgN = dramp.tile([GROUP * S, D], fp8)
                kg = dramp.tile([GROUP * D, S], fp8)

                with tc.tile_pool(name="yp", bufs=1) as ypool:
                  with tc.tile_pool(name="qp", bufs=1) as qpool:
                    q8 = qpool.tile([P, NT, S], fp8)

                    with tc.tile_pool(name="hq", bufs=1) as hqp:
                        with (
                            tc.tile_pool(name="ln1tmp", bufs=1) as ln1tmp,
                            tc.tile_pool(name="ln1ps", bufs=1,
                                         space="PSUM") as ln1ps,
                        ):
                            h1 = ln_T(xt, "g1", "bt1", hqp, ln1tmp, ln1ps,
                                      "ln1")

                        # Q weights are fully preloaded on the (otherwise
                        # idle) GpSimd DMA queue, spread through the K and V
                        # phases. The collectives HOL-block every DMA queued
                        # behind them, so Q must not need DMA after the
                        # gathers trigger.
                        # prefetch only the first half of Q's weights: the
                        # later j's compute after the gather window closes,
                        # so they can stream just-in-time on the sync queue
                        NQPRE = NT // 2
                        qw, qgw = {}, {}
                        for j in range(NQPRE):
                            qw[j] = hqp.tile([P, NT // 2, 2, P], fp8,
                                             tag="wqpre", bufs=NQPRE,
                                             name=f"wq_{j}")
                            qgw[j] = hqp.tile([P, NT // 2, 2, P], fp8,
                                              tag="wgqpre", bufs=NQPRE,
                                              name=f"wgq_{j}")

                        # V weight tiles: prefetched on the gpsimd queue
                        # during the K phase (V compute consumes them first;
                        # Q's prefetch follows during the V phase)
                        NP = NT // 2  # 8 contraction pairs
                        vw, vgw = {}, {}
                        for n in range(4):
                            for c in range(NP):
                                vw[n, c] = hqp.tile(
                                    [P, 2, S], fp8, tag="wv", bufs=32,
                                    name=f"wv_{n}_{c}")
                                vgw[n, c] = hqp.tile(
                                    [P, 2, S], fp8, tag="wgv", bufs=32,
                                    name=f"wgv_{n}_{c}")

                        def vw_prefetch_step(i):
                            n, c = divmod(i // 2, NP)
                            t = vgw[n, c] if i % 2 else vw[n, c]
                            w = wd["WgvT"] if i % 2 else wd["WvT"]
                            nc.gpsimd.dma_start(
                                t[:], w.ap()[c, :, :, n * S:(n + 1) * S])

                        # ---- K projection (T-layout); its AllGather
                        # triggers as soon as k_bounce is complete, which is
                        # why V/Q weights must already be in flight/SBUF ----
                        with (
                            tc.tile_pool(name="wproj", bufs=1) as wpool,
                            tc.tile_pool(name="pjps", bufs=1,
                                         space="PSUM") as pjps,
                        ):
                            def k_epi(j, main, sig):
                                kv = wpool.tile([P, S], fp8, tag="kv_out",
                                                name=f"kv_k_{j}", bufs=8)
                                nc.vector.scalar_tensor_tensor(
                                    kv[:], main[:], bias["bk"][:, j:j + 1],
                                    sig[:], op0=ALU.add, op1=ALU.mult)
                                nc.scalar.dma_start(
                                    k_bounce[j * P:(j + 1) * P, :], kv[:])
                                for i in range(4 * j, 4 * j + 4):
                                    vw_prefetch_step(i)

                            proj_gated_f8(h1, NT, NT, "Wk", "Wgk", "bgk",
                                          1.0 / WS, wpool, pjps, k_epi)

                        # ---- V projection, N-layout, fp8 DoubleRow.
                        # All V (and Q) weights are fully prefetched on the
                        # gpsimd queue before the K-gather can trigger: once
                        # a collective's DMA descriptors are on the rings,
                        # every later-queued DMA waits for peer delivery
                        # (~30-55us), so V/Q must not need loads then. ----
                        with (
                            tc.tile_pool(name="wv", bufs=1) as wvp,
                            tc.tile_pool(name="vps", bufs=1,
                                         space="PSUM") as vps,
                        ):
                            # Q weights (first half) prefetch during V
                            for j in range(NQPRE):
                                nc.gpsimd.dma_start(qw[j][:],
                                                    wd["Wq"].ap()[j])
                                nc.gpsimd.dma_start(qgw[j][:],
                                                    wd["Wgq"].ap()[j])
                            for n in range(4):
                                vmain = [vps.tile([P, S], fp32,
                                                  tag="v_main", bufs=4,
                                                  name=f"vm_{n}_{m}")
                                         for m in range(4)]
                                vgate = [vps.tile([P, S], fp32,
                                                  tag="v_gate", bufs=4,
                                                  name=f"vg_{n}_{m}")
                                         for m in range(4)]
                                for c in range(NP):
                                    wvt = vw[n, c]
                                    wgvt = vgw[n, c]
                                    for m in range(4):
                                        nc.tensor.matmul(
                                            vmain[m][:],
                                            h1[:, 2 * c:2 * c + 2,
                                               m * P:(m + 1) * P],
                                            wvt[:],
                                            start=(c == 0), stop=False,
                                            perf_mode=DR)
                                    for m in range(4):
                                        nc.tensor.matmul(
                                            vgate[m][:],
                                            h1[:, 2 * c:2 * c + 2,
                                               m * P:(m + 1) * P],
                                            wgvt[:],
                                            start=(c == 0), stop=False,
                                            perf_mode=DR)
                                for m in range(4):
                                    nc.tensor.matmul(
                                        vmain[m][:], ones_row[:],
                                        bvrow[:, n * S:(n + 1) * S],
                                        start=False, stop=True,
                                        skip_group_check=True)
                                    nc.tensor.matmul(
                                        vgate[m][:], ones_row[:],
                                        bgvrow[:, n * S:(n + 1) * S],
                                        start=False, stop=True,
                                        skip_group_check=True)
                                    vsig = wvp.tile([P, S], bf16,
                                                    tag="vsig", bufs=3,
                                                    name=f"vsig_{n}_{m}")
                                    nc.scalar.activation(vsig[:],
                                                         vgate[m][:],
                                                         AF.Sigmoid,
                                                         scale=1.0 / WS)
                                    vout = wvp.tile([P, S], fp8,
                                                    tag="vout", bufs=16,
                                                    name=f"vout_{n}_{m}")
                                    nc.vector.scalar_tensor_tensor(
                                        vout[:], vmain[m][:], 1.0, vsig[:],
                                        op0=ALU.mult, op1=ALU.mult)
                                    nc.scalar.dma_start(
                                        vN_bounce[m * P:(m + 1) * P,
                                                  n * S:(n + 1) * S],
                                        vout[:])

                        nc.gpsimd.collective_compute(
                            "AllGather", ALU.bypass, ins=[k_bounce[:]],
                            outs=[kg[:]], replica_groups=RG)
                        nc.gpsimd.collective_compute(
                            "AllGather", ALU.bypass, ins=[vN_bounce[:]],
                            outs=[vgN[:]], replica_groups=RG)

                        # ---- Q projection (weights already in SBUF) ----
                        with (
                            tc.tile_pool(name="wprojq", bufs=1) as wpoolq,
                            tc.tile_pool(name="pjqps", bufs=1,
                                         space="PSUM") as pjqps,
                        ):
                            for j in range(NT):
                                main = pjqps.tile([P, S], fp32,
                                                  name=f"Wq_m{j}",
                                                  tag="pj_main", bufs=2)
                                gate = pjqps.tile([P, S], fp32,
                                                  name=f"Wq_g{j}",
                                                  tag="pj_gate", bufs=2)
                                if j < NQPRE:
                                    wt, wg = qw[j], qgw[j]
                                else:
                                    wt = wpoolq.tile([P, NT // 2, 2, P],
                                                     fp8, tag="wmain",
                                                     name=f"w_Wq_{j}",
                                                     bufs=3)
                                    nc.sync.dma_start(wt[:],
                                                      wd["Wq"].ap()[j])
                                    wg = wpoolq.tile([P, NT // 2, 2, P],
                                                     fp8, tag="wgate",
                                                     name=f"w_Wgq_{j}",
                                                     bufs=3)
                                    nc.sync.dma_start(wg[:],
                                                      wd["Wgq"].ap()[j])
                                for c in range(NT // 2):
                                    nc.tensor.matmul(
                                        main[:], wt[:, c, :, :],
                                        h1[:, 2 * c:2 * c + 2, :],
                                        start=(c == 0),
                                        stop=(c == NT // 2 - 1),
                                        perf_mode=DR)
                                for c in range(NT // 2):
                                    nc.tensor.matmul(
                                        gate[:], wg[:, c, :, :],
                                        h1[:, 2 * c:2 * c + 2, :],
                                        start=(c == 0),
                                        stop=(c == NT // 2 - 1),
                                        perf_mode=DR)
                                sig = wpoolq.tile([P, S], bf16, tag="sig",
                                                  name=f"sig_Wq_{j}",
                                                  bufs=3)
                                nc.scalar.activation(
                                    sig[:], gate[:], AF.Sigmoid,
                                    bias=bias["bgq"][:, j:j + 1],
                                    scale=1.0 / WS)
                                nc.vector.scalar_tensor_tensor(
                                    q8[:, j, :], main[:],
                                    bias["bq"][:, j:j + 1],
                                    sig[:], op0=ALU.add, op1=ALU.mult)

                    # ---- phase B: attention (all fp8) ----
                    with (
                        tc.tile_pool(name="vres", bufs=1) as vresp,
                        tc.tile_pool(name="kstream", bufs=2) as kpool,
                        tc.tile_pool(name="apool", bufs=4) as apool,
                        tc.tile_pool(name="atps", bufs=1,
                                     space="PSUM") as atps,
                    ):
                        y8 = ypool.tile([P, NT, S], fp8)
                        Vt = vresp.tile([P, NKB, D], fp8)
                        for kb in range(NKB):
                            nc.gpsimd.dma_start(
                                Vt[:, kb, :],
                                vgN[kb * P:(kb + 1) * P, :])

                        head_state = {}

                        def finalize_head(h, Zp_h, Yp_h):
                            urow = rows.tile([1, S], fp32, name=f"u_{h}",
                                             tag="urow", bufs=2)
                            nc.vector.reciprocal_approx_fast(urow[:],
                                                             Zp_h[:])
                            ubf = rows.tile([1, S], bf16, name=f"ubf_{h}",
                                            tag="ubf", bufs=2)
                            nc.vector.tensor_copy(ubf[:], urow[:])
                            Us = apool.tile([P, S], bf16, tag="Us",
                                            name=f"Us_{h}", bufs=2)
                            nc.gpsimd.partition_broadcast(Us[:], ubf[:])
                            nc.vector.scalar_tensor_tensor(
                                y8[:, h, :], Yp_h[:], 1.0, Us[:],
                                op0=ALU.mult, op1=ALU.mult)

                        for hh in range(H):
                            # Kh loads ride the sync queue: the gpsimd
                            # queue is blocked until the V AllGather
                            # finishes, but kg is ready much earlier
                            Kh = kpool.tile([P, NKB * P], fp8, tag="Kh",
                                            name=f"Kh_{hh}")
                            for s_ in range(GROUP):
                                nc.sync.dma_start(
                                    Kh[:, s_ * S:(s_ + 1) * S],
                                    kg[s_ * D + hh * P:
                                       s_ * D + (hh + 1) * P, :])
                            Zp = atps.tile([1, S], fp32, name=f"Z_{hh}",
                                           tag="Zp", bufs=2)
                            Yp = atps.tile([P, S], fp32, name=f"Y_{hh}",
                                           tag="Yp", bufs=2)
                            ats = {}

                            def do_pair(c, hh=hh, Kh=Kh, ats=ats):
                                Lp2 = atps.tile([P, 2, S], fp32,
                                                name=f"L_{hh}_{c}",
                                                tag="logits", bufs=2)
                                for i in range(2):
                                    kb = 2 * c + i
                                    nc.tensor.matmul(
                                        Lp2[:, i, :],
                                        Kh[:, kb * P:(kb + 1) * P],
                                        q8[:, hh, :])
                                At2 = apool.tile(
                                    [P, 2, S], fp8, tag="At",
                                    name=f"At_{hh}_{c}", bufs=4)
                                nc.scalar.activation(
                                    At2[:], Lp2[:],
                                    AF.Exp, scale=ISCALE / (SA * SA))
                                ats[c] = At2

                            do_pair(0)
                            do_pair(1)
                            for c in range(NKB // 2):
                                if c + 2 < NKB // 2:
                                    do_pair(c + 2)
                                At2 = ats[c]
                                nc.tensor.matmul(Zp[:],
                                                 ones_z[:, :, 0:1],
                                                 At2[:],
                                                 start=(c == 0),
                                                 stop=(c == NKB // 2 - 1),
                                                 perf_mode=DR)
                                nc.tensor.matmul(
                                    Yp[:],
                                    Vt[:, 2 * c:2 * c + 2,
                                       hh * P:(hh + 1) * P],
                                    At2[:],
                                    start=(c == 0),
                                    stop=(c == NKB // 2 - 1),
                                    perf_mode=DR)
                                if c == 2 and hh > 0:
                                    finalize_head(hh - 1,
                                                  *head_state[hh - 1])
                            head_state[hh] = (Zp, Yp)
                        finalize_head(H - 1, *head_state[H - 1])

                  # ---- phase C: o-proj + residual (fp8, y at scale 32);
                  # LN2 stats accumulate per-j right here so phase D's
                  # LayerNorm only has the row chain + affine left ----
                  x2 = x2p.tile([P, NT, S], bf16, name="x2")
                  ln2S1s = rows.tile([1, S], fp32, name="ln2_S1s")
                  ln2S2s = rows.tile([1, S], fp32, name="ln2_S2s")
                  with (
                      tc.tile_pool(name="wproj2", bufs=1) as wpool2,
                      tc.tile_pool(name="pj2ps", bufs=1,
                                   space="PSUM") as pj2ps,
                  ):
                      ln2S1 = pj2ps.tile([1, S], fp32, name="ln2_S1")
                      ln2S2 = pj2ps.tile([1, S], fp32, name="ln2_S2")
                      def o_epi(j, main, sig):
                          tmp = wpool2.tile([P, S], fp32, tag="o_tmp",
                                            name=f"o_tmp_{j}", bufs=3)
                          nc.vector.scalar_tensor_tensor(
                              tmp[:], main[:], bias["bo"][:, j:j + 1],
                              sig[:], op0=ALU.add, op1=ALU.mult)
                          nc.vector.scalar_tensor_tensor(
                              x2[:, j, :], tmp[:], 1.0 / (SA * WS),
                              xt[:, j, :], op0=ALU.mult, op1=ALU.add)
                          ln_stats(x2[:, j, :], j, ln2S1, ln2S2, wpool2,
                                   "ln2")

                      proj_gated_f8(y8, NT, NT, "Wo", "Wgo", "bgo",
                                    1.0 / (SA * WS), wpool2, pj2ps, o_epi)
                      nc.vector.tensor_copy(ln2S1s[:], ln2S1[:])
                      nc.vector.tensor_copy(ln2S2s[:], ln2S2[:])

              # ---- phase D: LN2 + MLP (mains bf16, gates fp8); one psum
              # pool spans both MLP stages so the ring never drains ----
              with (
                  tc.tile_pool(name="midp", bufs=1) as midp,
                  tc.tile_pool(name="mlpps", bufs=1, space="PSUM") as mlpps,
              ):
                  mid = midp.tile([P, NF, S], bf16)
                  mid8 = midp.tile([P, NF, S], fp8)
                  with tc.tile_pool(name="h2p", bufs=1) as h2p:
                      with (
                          tc.tile_pool(name="ln2tmp", bufs=1) as ln2tmp,
                          tc.tile_pool(name="ln2ps", bufs=1,
                                       space="PSUM") as ln2ps,
                      ):
                          h2 = ln_T(x2, "g2", "bt2", h2p, ln2tmp, ln2ps,
                                    "ln2", stats=(ln2S1s, ln2S2s))
                      h2q = h2p.tile([P, NT, S], fp8, name="h2q")
                      for t in range(NT):
                          nc.scalar.activation(h2q[:, t, :], h2[:, t, :],
                                               AF.Identity)

                      with tc.tile_pool(name="wmlp1", bufs=1) as wm1:
                          # main (bf16) accumulation + fp8 gate; epilogue:
                          # t1 = (z + b) * sig; e = erf(t1 / sqrt2);
                          # mid = (e + 1) * t1 = 2*gelu (0.5 folded into
                          # W_out/Wg_out host-side); mid8 = fp8 copy.
                          for j in range(NF):
                              main = mlpps.tile([P, S], fp32,
                                               name=f"Win_m{j}",
                                               tag="pj_main", bufs=2)
                              gate = mlpps.tile([P, S], fp32,
                                               name=f"Win_g{j}",
                                               tag="pj_gate", bufs=2)
                              wt = wm1.tile([P, NT, P], bf16, tag="wmain",
                                            name=f"w_Win_{j}", bufs=3)
                              nc.sync.dma_start(wt[:], wd["Win"].ap()[j])
                              for t in range(NT):
                                  nc.tensor.matmul(main[:], wt[:, t, :],
                                                   h2[:, t, :],
                                                   start=(t == 0),
                                                   stop=(t == NT - 1))
                              wg = wm1.tile([P, NT // 2, 2, P], fp8,
                                            tag="wgate",
                                            name=f"w_Wgin_{j}", bufs=3)
                              nc.sync.dma_start(wg[:], wd["Wgin"].ap()[j])
                              for c in range(NT // 2):
                                  nc.tensor.matmul(
                                      gate[:], wg[:, c, :, :],
                                      h2q[:, 2 * c:2 * c + 2, :],
                                      start=(c == 0),
                                      stop=(c == NT // 2 - 1),
                                      perf_mode=DR)
                              sig = wm1.tile([P, S], bf16, tag="sig",
                                             name=f"sig_Win_{j}", bufs=3)
                              nc.scalar.activation(
                                  sig[:], gate[:], AF.Sigmoid,
                                  bias=bias["bgin"][:, j:j + 1],
                                  scale=1.0 / WS)
                              t1 = wm1.tile([P, S], bf16, tag="mid_t1",
                                            name=f"t1_{j}", bufs=3)
                              nc.vector.scalar_tensor_tensor(
                                  t1[:], main[:],
                                  bias["bin"][:, j:j + 1], sig[:],
                                  op0=ALU.add, op1=ALU.mult)
                              er = wm1.tile([P, S], bf16, tag="mid_er",
                                            name=f"er_{j}", bufs=3)
                              nc.scalar.activation(er[:], t1[:], AF.Erf,
                                                   scale=RSQRT2)
                              nc.vector.scalar_tensor_tensor(
                                  mid[:, j, :], er[:], 1.0, t1[:],
                                  op0=ALU.add, op1=ALU.mult)
                              nc.scalar.activation(mid8[:, j, :],
                                                   mid[:, j, :],
                                                   AF.Identity)

                  with tc.tile_pool(name="wmlp2", bufs=1) as wm2:
                      # W_out main bf16 (x0.5 host), gate fp8 (x32 host)
                      for j in range(NT):
                          main = mlpps.tile([P, S], fp32,
                                           name=f"Wout_m{j}",
                                           tag="pj_main", bufs=2)
                          gate = mlpps.tile([P, S], fp32,
                                           name=f"Wout_g{j}",
                                           tag="pj_gate", bufs=2)
                          for ci in range(2):
                              wt = wm2.tile([P, 32, P], bf16, tag="wmain",
                                            name=f"w_Wout_{j}_{ci}",
                                            bufs=2)
                              nc.sync.dma_start(
                                  wt[:],
                                  wd["Wout"].ap()[j, :,
                                                  32 * ci:32 * (ci + 1),
                                                  :])
                              for tt_ in range(32):
                                  t = 32 * ci + tt_
                                  nc.tensor.matmul(main[:], wt[:, tt_, :],
                                                   mid[:, t, :],
                                                   start=(t == 0),
                                                   stop=(t == NF - 1))
                          for ci in range(2):
                              wg = wm2.tile([P, 16, 2, P], fp8,
                                            tag="wgate",
                                            name=f"w_Wgout_{j}_{ci}",
                                            bufs=2)
                              nc.sync.dma_start(
                                  wg[:],
                                  wd["Wgout"].ap()[j, :,
                                                   16 * ci:16 * (ci + 1),
                                                   :, :])
                              for cc in range(16):
                                  c = 16 * ci + cc
                                  nc.tensor.matmul(
                                      gate[:], wg[:, cc, :, :],
                                      mid8[:, 2 * c:2 * c + 2, :],
                                      start=(c == 0),
                                      stop=(c == NF // 2 - 1),
                                      perf_mode=DR)
                          sig = wm2.tile([P, S], bf16, tag="sig",
                                         name=f"sig_Wout_{j}", bufs=3)
                          nc.scalar.activation(
                              sig[:], gate[:], AF.Sigmoid,
                              bias=bias["bgout"][:, j:j + 1],
                              scale=1.0 / WS)
                          tmp = wm2.tile([P, S], fp32, tag="out_tmp",
                                         name=f"out_tmp_{j}", bufs=3)
                          nc.vector.scalar_tensor_tensor(
                              tmp[:], main[:], bias["bout"][:, j:j + 1],
                              sig[:], op0=ALU.add, op1=ALU.mult)
                          outf = wm2.tile([P, S], fp32, tag="out_f",
                                          name=f"out_f_{j}", bufs=3)
                          nc.vector.tensor_tensor(outf[:], tmp[:],
                                                  x2[:, j, :], op=ALU.add)
                          nc.sync.dma_start(
                              out_d.ap()[j * P:(j + 1) * P, :], outf[:])

    nc.compile()
    return nc


def _prep_shared_inputs(inputs):
    m = {}
    for nm, w, sc in (("Wq", "W_q", SA), ("Wgq", "Wg_q", WS),
                      ("Wk", "W_k", SA), ("Wgk", "Wg_k", WS),
                      ("Wo", "W_o", WS), ("Wgo", "Wg_o", WS)):
        m[nm] = _w_tiled_f8(np.asarray(inputs[w]), sc)
    m["Wgin"] = _w_tiled_f8(np.asarray(inputs["Wg_in"]), WS)
    m["Wgout"] = _w_tiled_f8(np.asarray(inputs["Wg_out"]), WS / 2.0)
    m["Win"] = _w_tiled_bf(np.asarray(inputs["W_in"]), 1.0)
    m["Wout"] = _w_tiled_bf(np.asarray(inputs["W_out"]), 0.5)
    m["WvT"] = _wT_pairs_f8(np.asarray(inputs["W_v"]), SA)
    m["WgvT"] = _wT_pairs_f8(np.asarray(inputs["Wg_v"]), WS)
    m["bvrow"] = (np.asarray(inputs["b_v"]) * SA).astype(_BF).reshape(1, D)
    m["bgvrow"] = (np.asarray(inputs["bg_v"]) * WS).astype(_BF).reshape(1, D)
    for nm, bn, sc in (("bq", "b_q", SA), ("bgq", "bg_q", 1.0),
                       ("bk", "b_k", SA), ("bgk", "bg_k", 1.0),
                       ("bo", "b_o", SA * WS), ("bgo", "bg_o", 1.0),
                       ("bin", "b_in", 1.0), ("bgin", "bg_in", 1.0),
                       ("bout", "b_out", 1.0), ("bgout", "bg_out", 1.0),
                       ("g1", "ln1_g", 1.0), ("bt1", "ln1_b", 1.0),
                       ("g2", "ln2_g", 1.0), ("bt2", "ln2_b", 1.0)):
        m[nm] = _b_cols(np.asarray(inputs[bn]), sc)
    return m


def _install_trace_shim():
    """Provide antenv.axon_hooks (NTFF profiling) if the image lacks it."""
    import contextlib
    import ctypes
    import types

    try:
        import antenv.axon_hooks  # noqa: F401
        return
    except ImportError:
        pass
    try:
        import antenv
    except ImportError:
        return
    so_path = "/opt/axon/libaxon_pjrt.so"
    try:
        lib = ctypes.CDLL(so_path)
    except OSError:
        return
    if not hasattr(lib, "axon_start_nrt_profile"):
        return
    lib.axon_start_nrt_profile.argtypes = [ctypes.POINTER(ctypes.c_int64),
                                           ctypes.c_size_t]
    lib.axon_start_nrt_profile.restype = ctypes.c_int64
    lib.axon_stop_nrt_profile.argtypes = [ctypes.c_char_p]
    lib.axon_stop_nrt_profile.restype = ctypes.c_int64

    @contextlib.contextmanager
    def hook(output_dir, device_ids):
        import jax

        jax.devices()
        if device_ids:
            ids = (ctypes.c_int64 * len(device_ids))(*device_ids)
            rc = lib.axon_start_nrt_profile(ids, len(device_ids))
        else:
            rc = lib.axon_start_nrt_profile(None, 0)
        if rc != 0:
            raise RuntimeError(f"axon_start_nrt_profile rc={rc}")
        try:
            yield
        finally:
            n = lib.axon_stop_nrt_profile(str(output_dir).encode())
            print(f"profile: {n} ntff file(s) in {output_dir}",
                  file=sys.stderr)

    mod = types.ModuleType("antenv.axon_hooks")
    mod.get_axon_ntff_profile_hook = lambda: hook
    mod.set_axon_ntff_profile_hook = lambda h: None
    sys.modules["antenv.axon_hooks"] = mod
    antenv.axon_hooks = mod


LAST_RESULTS = None


def kernel(_trace=False, **inputs):
    global _COMPILED, LAST_RESULTS
    from concourse import bass_utils

    if _trace:
        _install_trace_shim()

    if _COMPILED is None:
        _COMPILED = _build()
    nc = _COMPILED

    shared = _prep_shared_inputs(inputs)
    x = np.asarray(inputs["x"], dtype=np.float32)  # [B, T, D]
    in_maps = []
    for c in range(N_CORES):
        g, s = divmod(c, GROUP)
        xT_c = np.ascontiguousarray(x[g, s * S:(s + 1) * S, :].T.astype(_BF))
        m = dict(shared)
        m["xT"] = xT_c
        in_maps.append(m)

    LAST_RESULTS = bass_utils.run_bass_kernel_spmd(
        nc, in_maps, core_ids=list(range(N_CORES)), trace=_trace)

    out = np.empty((B, T, D), dtype=np.float32)
    for c in range(N_CORES):
        g, s = divmod(c, GROUP)
        out[g, s * S:(s + 1) * S, :] = LAST_RESULTS.results[c]["outT"].T
    return out


# revision 54
# speedup vs baseline: 1.0986x; 1.0954x over previous
"""Trainium2 Bass kernel for nn_DGEBlock (dense transformer block with
MoE-gated linears), distributed over 8 NeuronCores.

Sharding: data-parallel over batch (2 groups of 4 cores) x sequence-parallel
over tokens within each batch (512 tokens per core). Weights replicated.
Activations live feature-major ("T-layout": [d, tok]) in SBUF; V is
projected token-major (N-layout) so attention PV needs no transposes.

Precision plan (tolerance 2e-2): fp8-e4m3 DoubleRow matmuls (2 k-tiles per
instruction, ~2x PE throughput) for all six sigmoid-gate projections, the
q/k/v/o mains and attention PV/Z; plain fp8 for QK; bf16 for the two MLP
mains (their error passes unsquashed into the residual; measured sim error
for this mix is 1.1e-2 vs 1.9e-2 with an MLP main in fp8). Weights are
pre-scaled on host (x64 gates, x32 qkv mains so no fp8 activation exceeds
TRN e4m3's +-240 -> Inf); epilogue scales fold everything back out. GELU is
computed via Erf (same ACT table set as Sigmoid -> no table thrash in the
MLP phase; the 0.5 folds into W_out host-side).

Scheduling: K/V are gathered as fp8. A collective's DMA descriptors
head-of-line block every DMA queued behind them until all peers deliver
(~30-55us of skew), so the V and first-half-Q weights are fully prefetched
into SBUF on the gpsimd queue before the K gather can trigger, and the
attention Kh loads ride the sync queue (the gpsimd queue is parked on the
V gather). LN2 statistics accumulate inside the o-projection epilogues;
paired logit-EXPs ([128,2,512] PSUM reads) halve ACT instruction count in
attention; softmax 1/Z runs via reciprocal_approx_fast + a gpsimd
partition_broadcast instead of a PE broadcast matmul.
"""

import sys

for _p in ("/opt/trn_rl_repo",):
    if _p not in sys.path:
        sys.path.append(_p)

import numpy as np
import ml_dtypes

# ---------------------------------------------------------------- constants
B = 2
T = 2048
D = 2048
H = 16
HD = 128
FF = 4 * D  # 8192
EPS = 1e-5

N_CORES = 8
GROUP = 4  # cores per batch group (sequence-parallel degree)
S = T // GROUP  # tokens per core = 512
P = 128
NT = D // P  # 16 feature tiles
NF = FF // P  # 64 hidden tiles
NKB = T // P  # 16 key blocks per batch
ISCALE = 1.0 / float(np.sqrt(HD))

WS = 64.0  # fp8 weight pre-scale (gate paths)
SA = 32.0  # q/k/v main-path weight pre-scale: keeps the fp8 activations
           # (32*k etc., |.| <~ 160) inside TRN e4m3's +-240 range
RSQRT2 = 1.0 / float(np.sqrt(2.0))

RG = [[0, 1, 2, 3], [4, 5, 6, 7]]

_BF = ml_dtypes.bfloat16
_F8 = ml_dtypes.float8_e4m3  # IEEE e4m3, max 240 == TRN FP8_EXP4

_COMPILED = None


# ------------------------------------------------------------- host prep
def _w_tiled_bf(W, scale=1.0):
    """W [dout, din] -> [nj, 128, nt, 128] bf16: W^T tile (t, j) layout;
    [j, p, t, jc] == W[j*128+jc, t*128+p]."""
    dout, din = W.shape
    nj, nt = dout // P, din // P
    return np.ascontiguousarray(
        (W.reshape(nj, P, nt, P) * scale).transpose(0, 3, 2, 1).astype(_BF)
    )


def _w_tiled_f8(W, scale=WS):
    """W [dout, din] -> [nj, 128, nt//2, 2, 128] fp8 DoubleRow pairs:
    [j, p, c, i, jc] == scale * W[j*128+jc, (2c+i)*128+p]."""
    dout, din = W.shape
    nj, nt = dout // P, din // P
    w = (W.reshape(nj, P, nt // 2, 2, P) * scale).transpose(0, 4, 2, 3, 1)
    w = np.clip(w, -240.0, 240.0)
    return np.ascontiguousarray(w.astype(_F8))


def _wT_pairs_f8(W, scale=WS):
    """W [dout, din] -> W^T pair layout [din//256, 128, 2, dout] fp8:
    [c, p, i, col] == scale * W[col, (2c+i)*128+p] (N-layout rhs)."""
    dout, din = W.shape
    w = (W.T.reshape(din // 256, 2, P, dout) * scale).transpose(0, 2, 1, 3)
    w = np.clip(w, -240.0, 240.0)
    return np.ascontiguousarray(w.astype(_F8))


def _b_cols(b, scale=1.0):
    """b [dout] -> [128, nj] fp32: column j holds b[j*128:(j+1)*128]."""
    nj = b.shape[0] // P
    return np.ascontiguousarray((b.reshape(nj, P) * scale).T.astype(np.float32))


# ------------------------------------------------------------- device build
def _build():
    from concourse import bacc, tile, mybir

    fp32 = mybir.dt.float32
    bf16 = mybir.dt.bfloat16
    fp8 = mybir.dt.float8e4
    AF = mybir.ActivationFunctionType
    ALU = mybir.AluOpType
    DR = mybir.MatmulPerfMode.DoubleRow

    nc = bacc.Bacc("TRN2", target_bir_lowering=False, debug=False,
                   num_devices=N_CORES)

    # ---- I/O tensors
    xT_d = nc.dram_tensor("xT", [D, S], bf16, kind="ExternalInput")
    wd = {}
    # fp8 DoubleRow pair weights (scale x64): qkvo mains + all gates
    for nm in ("Wq", "Wgq", "Wk", "Wgk", "Wo", "Wgo"):
        wd[nm] = nc.dram_tensor(nm, [NT, P, NT // 2, 2, P], fp8,
                                kind="ExternalInput")
    wd["Wgin"] = nc.dram_tensor("Wgin", [NF, P, NT // 2, 2, P], fp8,
                                kind="ExternalInput")
    wd["Wgout"] = nc.dram_tensor("Wgout", [NT, P, NF // 2, 2, P], fp8,
                                 kind="ExternalInput")
    wd["Win"] = nc.dram_tensor("Win", [NF, P, NT // 2, 2, P], fp8,
                               kind="ExternalInput")
    # bf16 main for the MLP output projection (accuracy-critical)
    wd["Wout"] = nc.dram_tensor("Wout", [NT, P, NF, P], bf16,
                                kind="ExternalInput")
    # V projection runs in N-layout: W^T fp8 pair layout + bias rows (x64)
    wd["WvT"] = nc.dram_tensor("WvT", [NT // 2, P, 2, D], fp8,
                               kind="ExternalInput")
    wd["WgvT"] = nc.dram_tensor("WgvT", [NT // 2, P, 2, D], fp8,
                                kind="ExternalInput")
    bvrow_d = nc.dram_tensor("bvrow", [1, D], bf16, kind="ExternalInput")
    bgvrow_d = nc.dram_tensor("bgvrow", [1, D], bf16, kind="ExternalInput")
    bd = {}
    for nm in ("bq", "bgq", "bk", "bgk", "bo", "bgo",
               "bout", "bgout", "g1", "bt1", "g2", "bt2"):
        bd[nm] = nc.dram_tensor(nm, [P, NT], fp32, kind="ExternalInput")
    for nm in ("bin", "bgin"):
        bd[nm] = nc.dram_tensor(nm, [P, NF], fp32, kind="ExternalInput")
    out_d = nc.dram_tensor("outT", [D, S], fp32, kind="ExternalOutput")

    with tile.TileContext(nc) as tc:
        with (
            tc.tile_pool(name="const", bufs=1) as constp,
            tc.tile_pool(name="bias", bufs=1) as biasp,
            tc.tile_pool(name="rows", bufs=1) as rows,
            tc.tile_pool(name="dram", bufs=1, space="DRAM") as dramp,
        ):
            ones_col = constp.tile([P, 1], bf16)
            nc.vector.memset(ones_col[:], 1.0)
            ones_row = constp.tile([1, P], bf16)
            nc.vector.memset(ones_row[:], 1.0)
            # fp8 ones for DoubleRow Z-sums; padded so the pair-dim byte
            # stride is 16 (DoubleRow AP constraint)
            ones_z = constp.tile([P, 2, 16], fp8)
            nc.vector.memset(ones_z[:], 1.0)
            eps_t = constp.tile([1, 1], fp32)
            nc.vector.memset(eps_t[:], EPS)
            # bias/row constants load via the scalar queue so the sync
            # queue starts streaming x immediately
            bvrow = constp.tile([1, D], bf16)
            nc.scalar.dma_start(bvrow[:], bvrow_d.ap())
            bgvrow = constp.tile([1, D], bf16)
            nc.scalar.dma_start(bgvrow[:], bgvrow_d.ap())

            bias = {}
            for nm in bd:
                ncols = NF if nm in ("bin", "bgin") else NT
                btile = biasp.tile([P, ncols], fp32, name=f"bias_{nm}")
                nc.scalar.dma_start(btile[:], bd[nm].ap())
                bias[nm] = btile

            # ---------- helpers ----------
            def ln_stats(src_j, j, S1, S2, sqpool, name):
                """One tile's contribution to LN stats (callable per-j from
                an earlier phase's epilogue to hide the latency)."""
                sq = sqpool.tile([P, S], bf16, name=f"{name}_sq{j}",
                                 tag=f"{name}_sq", bufs=3)
                nc.scalar.activation(sq[:], src_j, AF.Square)
                nc.tensor.matmul(S1[:], ones_col[:], src_j,
                                 start=(j == 0), stop=(j == NT - 1))
                nc.tensor.matmul(S2[:], ones_col[:], sq[:],
                                 start=(j == 0), stop=(j == NT - 1))

            def ln_T(src, gname, bname, hpool, tmpool, psln, name,
                     stats=None):
                """LayerNorm over features of a T-layout activation.

                src: SBUF [128, NT, S] bf16 -> [128, NT, S] (dtype per
                hpool_dtype). Stats via ones-matmuls (or passed in via
                `stats` if accumulated earlier); per-token scale/shift rows
                broadcast via rank-1 matmuls; bf16 DVE affine (2x rate).
                """
                if stats is None:
                    S1 = psln.tile([1, S], fp32, name=f"{name}_S1",
                                   tag="ln_S1")
                    S2 = psln.tile([1, S], fp32, name=f"{name}_S2",
                                   tag="ln_S2")
                    for t in range(NT):
                        ln_stats(src[:, t, :], t, S1, S2, tmpool, name)
                else:
                    S1, S2 = stats

                def row(nm, dt=fp32):
                    return rows.tile([1, S], dt, name=f"{name}_{nm}",
                                     tag=f"ln_{nm}")

                mean = row("mean")
                nc.vector.tensor_scalar_mul(mean[:], S1[:], 1.0 / D)
                m2 = row("m2")
                nc.vector.tensor_scalar_mul(m2[:], S2[:], 1.0 / D)
                msq = row("msq")
                nc.vector.tensor_tensor(msq[:], mean[:], mean[:],
                                        op=ALU.mult)
                var = row("var")
                nc.vector.tensor_tensor(var[:], m2[:], msq[:],
                                        op=ALU.subtract)
                std = row("std")
                nc.scalar.activation(std[:], var[:], AF.Sqrt,
                                     bias=eps_t[:])
                rstd = row("rstd")
                nc.vector.reciprocal_approx_fast(rstd[:], std[:])
                rstd_bf = row("rstdbf", bf16)
                nc.vector.tensor_copy(rstd_bf[:], rstd[:])
                mr_bf = row("mrbf", bf16)
                nc.vector.tensor_tensor(mr_bf[:], mean[:], rstd[:],
                                        op=ALU.mult)
                Ab_p = psln.tile([P, S], fp32, name=f"{name}_Abp",
                                 tag="ln_Abp")
                nc.tensor.matmul(Ab_p[:], ones_row[:], rstd_bf[:])
                Bb_p = psln.tile([P, S], fp32, name=f"{name}_Bbp",
                                 tag="ln_Bbp")
                nc.tensor.matmul(Bb_p[:], ones_row[:], mr_bf[:])
                Ab = tmpool.tile([P, S], bf16, name=f"{name}_Ab")
                nc.vector.tensor_copy(Ab[:], Ab_p[:])
                Bb = tmpool.tile([P, S], bf16, name=f"{name}_Bb")
                nc.vector.tensor_copy(Bb[:], Bb_p[:])
                h = hpool.tile([P, NT, S], hpool_dtype[name],
                               name=f"{name}_h")
                for t in range(NT):
                    tmp = tmpool.tile([P, S], bf16, name=f"{name}_t0_{t}",
                                      tag="ln_t0", bufs=6)
                    nc.vector.tensor_tensor(tmp[:], src[:, t, :], Ab[:],
                                            op=ALU.mult)
                    tmp2 = tmpool.tile([P, S], bf16, name=f"{name}_t1_{t}",
                                       tag="ln_t1", bufs=6)
                    nc.vector.tensor_tensor(tmp2[:], tmp[:], Bb[:],
                                            op=ALU.subtract)
                    nc.scalar.activation(h[:, t, :], tmp2[:], AF.Identity,
                                         bias=bias[bname][:, t:t + 1],
                                         scale=bias[gname][:, t:t + 1])
                return h

            hpool_dtype = {"ln1": fp8, "ln2": fp8}

            def proj_gated_f8(src, nt, nj, wname, wgname, bgname, sig_scale,
                              wpool, pspool, epilogue, cchunk=None, wbufs=3):
                """Gated projection, T-layout, fp8 DoubleRow: for each output
                tile j, main/gate = sum_c W8(c,j).T @ src[:, 2c:2c+2, :],
                then epilogue(j, main_psum, sig_sbuf). src fp8 [P, nt, S]."""
                npair = nt // 2
                if cchunk is None:
                    cchunk = npair
                nchunk = npair // cchunk
                for j in range(nj):
                    main = pspool.tile([P, S], fp32, name=f"{wname}_m{j}",
                                       tag="pj_main", bufs=2)
                    gate = pspool.tile([P, S], fp32, name=f"{wname}_g{j}",
                                       tag="pj_gate", bufs=2)
                    for ci in range(nchunk):
                        wt = wpool.tile([P, cchunk, 2, P], fp8, tag="wmain",
                                        name=f"w_{wname}_{j}_{ci}",
                                        bufs=wbufs)
                        nc.sync.dma_start(
                            wt[:],
                            wd[wname].ap()[j, :,
                                           ci * cchunk:(ci + 1) * cchunk,
                                           :, :])
                        for cc in range(cchunk):
                            c = ci * cchunk + cc
                            nc.tensor.matmul(main[:], wt[:, cc, :, :],
                                             src[:, 2 * c:2 * c + 2, :],
                                             start=(c == 0),
                                             stop=(c == npair - 1),
                                             perf_mode=DR)
                    for ci in range(nchunk):
                        wg = wpool.tile([P, cchunk, 2, P], fp8, tag="wgate",
                                        name=f"w_{wgname}_{j}_{ci}",
                                        bufs=wbufs)
                        nc.sync.dma_start(
                            wg[:],
                            wd[wgname].ap()[j, :,
                                            ci * cchunk:(ci + 1) * cchunk,
                                            :, :])
                        for cc in range(cchunk):
                            c = ci * cchunk + cc
                            nc.tensor.matmul(gate[:], wg[:, cc, :, :],
                                             src[:, 2 * c:2 * c + 2, :],
                                             start=(c == 0),
                                             stop=(c == npair - 1),
                                             perf_mode=DR)
                    sig = wpool.tile([P, S], bf16, tag="sig",
                                     name=f"sig_{wname}_{j}", bufs=3)
                    nc.scalar.activation(sig[:], gate[:], AF.Sigmoid,
                                         bias=bias[bgname][:, j:j + 1],
                                         scale=sig_scale)
                    epilogue(j, main, sig)

            # x2 outlives phases A-C (used by LN2 + MLP residual)
            with tc.tile_pool(name="x2p", bufs=1) as x2p:
              with tc.tile_pool(name="xt", bufs=1) as xtp:
                xt = xtp.tile([P, NT, S], bf16)
                xT_v = xT_d.ap().rearrange("(t p) s -> t p s", p=P)
                for t in range(NT):
                    # split across two queues so two DMA rings fetch x in
                    # parallel (the gpsimd queue is idle this early)
                    eng = nc.sync if t % 2 == 0 else nc.gpsimd
                    eng.dma_start(xt[:, t, :], xT_v[t])

                vN_bounce = dramp.tile([S, D], fp8)
                k_bounce = dramp.tile([D, S], fp8)
                vgN = dramp.tile([GROUP * S, D], fp8)
                kg = dramp.tile([GROUP * D, S], fp8)

                with tc.tile_pool(name="yp", bufs=1) as ypool:
                  with tc.tile_pool(name="qp", bufs=1) as qpool:
                    q8 = qpool.tile([P, NT, S], fp8)

                    with tc.tile_pool(name="hq", bufs=1) as hqp:
                        with (
                            tc.tile_pool(name="ln1tmp", bufs=1) as ln1tmp,
                            tc.tile_pool(name="ln1ps", bufs=1,
                                         space="PSUM") as ln1ps,
                        ):
                            h1 = ln_T(xt, "g1", "bt1", hqp, ln1tmp, ln1ps,
                                      "ln1")

                        # Q weights are fully preloaded on the (otherwise
                        # idle) GpSimd DMA queue, spread through the K and V
                        # phases. The collectives HOL-block every DMA queued
                        # behind them, so Q must not need DMA after the
                        # gathers trigger.
                        # prefetch only the first half of Q's weights: the
                        # later j's compute after the gather window closes,
                        # so they can stream just-in-time on the sync queue
                        NQPRE = NT // 2
                        qw, qgw = {}, {}
                        for j in range(NQPRE):
                            qw[j] = hqp.tile([P, NT // 2, 2, P], fp8,
                                             tag="wqpre", bufs=NQPRE,
                                             name=f"wq_{j}")
                            qgw[j] = hqp.tile([P, NT // 2, 2, P], fp8,
                                              tag="wgqpre", bufs=NQPRE,
                                              name=f"wgq_{j}")

                        # V weight tiles: prefetched on the gpsimd queue
                        # during the K phase (V compute consumes them first;
                        # Q's prefetch follows during the V phase)
                        NP = NT // 2  # 8 contraction pairs
                        vw, vgw = {}, {}
                        for n in range(4):
                            for c in range(NP):
                                vw[n, c] = hqp.tile(
                                    [P, 2, S], fp8, tag="wv", bufs=32,
                                    name=f"wv_{n}_{c}")
                                vgw[n, c] = hqp.tile(
                                    [P, 2, S], fp8, tag="wgv", bufs=32,
                                    name=f"wgv_{n}_{c}")

                        def vw_prefetch_step(i):
                            n, c = divmod(i // 2, NP)
                            t = vgw[n, c] if i % 2 else vw[n, c]
                            w = wd["WgvT"] if i % 2 else wd["WvT"]
                            nc.gpsimd.dma_start(
                                t[:], w.ap()[c, :, :, n * S:(n + 1) * S])

                        # ---- K projection (T-layout); its AllGather
                        # triggers as soon as k_bounce is complete, which is
                        # why V/Q weights must already be in flight/SBUF ----
                        with (
                            tc.tile_pool(name="wproj", bufs=1) as wpool,
                            tc.tile_pool(name="pjps", bufs=1,
                                         space="PSUM") as pjps,
                        ):
                            def k_epi(j, main, sig):
                                kv = wpool.tile([P, S], fp8, tag="kv_out",
                                                name=f"kv_k_{j}", bufs=8)
                                nc.vector.scalar_tensor_tensor(
                                    kv[:], main[:], bias["bk"][:, j:j + 1],
                                    sig[:], op0=ALU.add, op1=ALU.mult)
                                nc.scalar.dma_start(
                                    k_bounce[j * P:(j + 1) * P, :], kv[:])
                                for i in range(4 * j, 4 * j + 4):
                                    vw_prefetch_step(i)

                            proj_gated_f8(h1, NT, NT, "Wk", "Wgk", "bgk",
                                          1.0 / WS, wpool, pjps, k_epi)

                        # ---- V projection, N-layout, fp8 DoubleRow.
                        # All V (and Q) weights are fully prefetched on the
                        # gpsimd queue before the K-gather can trigger: once
                        # a collective's DMA descriptors are on the rings,
                        # every later-queued DMA waits for peer delivery
                        # (~30-55us), so V/Q must not need loads then. ----
                        with (
                            tc.tile_pool(name="wv", bufs=1) as wvp,
                            tc.tile_pool(name="vps", bufs=1,
                                         space="PSUM") as vps,
                        ):
                            # Q weights (first half) prefetch during V
                            for j in range(NQPRE):
                                nc.gpsimd.dma_start(qw[j][:],
                                                    wd["Wq"].ap()[j])
                                nc.gpsimd.dma_start(qgw[j][:],
                                                    wd["Wgq"].ap()[j])
                            for n in range(4):
                                vmain = [vps.tile([P, S], fp32,
                                                  tag="v_main", bufs=4,
                                                  name=f"vm_{n}_{m}")
                                         for m in range(4)]
                                vgate = [vps.tile([P, S], fp32,
                                                  tag="v_gate", bufs=4,
                                                  name=f"vg_{n}_{m}")
                                         for m in range(4)]
                                for c in range(NP):
                                    wvt = vw[n, c]
                                    wgvt = vgw[n, c]
                                    for m in range(4):
                                        nc.tensor.matmul(
                                            vmain[m][:],
                                            h1[:, 2 * c:2 * c + 2,
                                               m * P:(m + 1) * P],
                                            wvt[:],
                                            start=(c == 0), stop=False,
                                            perf_mode=DR)
                                    for m in range(4):
                                        nc.tensor.matmul(
                                            vgate[m][:],
                                            h1[:, 2 * c:2 * c + 2,
                                               m * P:(m + 1) * P],
                                            wgvt[:],
                                            start=(c == 0), stop=False,
                                            perf_mode=DR)
                                for m in range(4):
                                    nc.tensor.matmul(
                                        vmain[m][:], ones_row[:],
                                        bvrow[:, n * S:(n + 1) * S],
                                        start=False, stop=True,
                                        skip_group_check=True)
                                    nc.tensor.matmul(
                                        vgate[m][:], ones_row[:],
                                        bgvrow[:, n * S:(n + 1) * S],
                                        start=False, stop=True,
                                        skip_group_check=True)
                                    vsig = wvp.tile([P, S], bf16,
                                                    tag="vsig", bufs=3,
                                                    name=f"vsig_{n}_{m}")
                                    nc.scalar.activation(vsig[:],
                                                         vgate[m][:],
                                                         AF.Sigmoid,
                                                         scale=1.0 / WS)
                                    vout = wvp.tile([P, S], fp8,
                                                    tag="vout", bufs=16,
                                                    name=f"vout_{n}_{m}")
                                    nc.vector.scalar_tensor_tensor(
                                        vout[:], vmain[m][:], 1.0, vsig[:],
                                        op0=ALU.mult, op1=ALU.mult)
                                    nc.scalar.dma_start(
                                        vN_bounce[m * P:(m + 1) * P,
                                                  n * S:(n + 1) * S],
                                        vout[:])

                        nc.gpsimd.collective_compute(
                            "AllGather", ALU.bypass, ins=[k_bounce[:]],
                            outs=[kg[:]], replica_groups=RG)
                        nc.gpsimd.collective_compute(
                            "AllGather", ALU.bypass, ins=[vN_bounce[:]],
                            outs=[vgN[:]], replica_groups=RG)

                        # ---- Q projection (weights already in SBUF) ----
                        with (
                            tc.tile_pool(name="wprojq", bufs=1) as wpoolq,
                            tc.tile_pool(name="pjqps", bufs=1,
                                         space="PSUM") as pjqps,
                        ):
                            for j in range(NT):
                                main = pjqps.tile([P, S], fp32,
                                                  name=f"Wq_m{j}",
                                                  tag="pj_main", bufs=2)
                                gate = pjqps.tile([P, S], fp32,
                                                  name=f"Wq_g{j}",
                                                  tag="pj_gate", bufs=2)
                                if j < NQPRE:
                                    wt, wg = qw[j], qgw[j]
                                else:
                                    wt = wpoolq.tile([P, NT // 2, 2, P],
                                                     fp8, tag="wmain",
                                                     name=f"w_Wq_{j}",
                                                     bufs=3)
                                    nc.sync.dma_start(wt[:],
                                                      wd["Wq"].ap()[j])
                                    wg = wpoolq.tile([P, NT // 2, 2, P],
                                                     fp8, tag="wgate",
                                                     name=f"w_Wgq_{j}",
                                                     bufs=3)
                                    nc.sync.dma_start(wg[:],
                                                      wd["Wgq"].ap()[j])
                                for c in range(NT // 2):
                                    nc.tensor.matmul(
                                        main[:], wt[:, c, :, :],
                                        h1[:, 2 * c:2 * c + 2, :],
                                        start=(c == 0),
                                        stop=(c == NT // 2 - 1),
                                        perf_mode=DR)
                                for c in range(NT // 2):
                                    nc.tensor.matmul(
                                        gate[:], wg[:, c, :, :],
                                        h1[:, 2 * c:2 * c + 2, :],
                                        start=(c == 0),
                                        stop=(c == NT // 2 - 1),
                                        perf_mode=DR)
                                sig = wpoolq.tile([P, S], bf16, tag="sig",
                                                  name=f"sig_Wq_{j}",
                                                  bufs=3)
                                nc.scalar.activation(
                                    sig[:], gate[:], AF.Sigmoid,
                                    bias=bias["bgq"][:, j:j + 1],
                                    scale=1.0 / WS)
                                nc.vector.scalar_tensor_tensor(
                                    q8[:, j, :], main[:],
                                    bias["bq"][:, j:j + 1],
                                    sig[:], op0=ALU.add, op1=ALU.mult)

                    # ---- phase B: attention (all fp8) ----
                    with (
                        tc.tile_pool(name="vres", bufs=1) as vresp,
                        tc.tile_pool(name="kstream", bufs=2) as kpool,
                        tc.tile_pool(name="apool", bufs=4) as apool,
                        tc.tile_pool(name="atps", bufs=1,
                                     space="PSUM") as atps,
                    ):
                        y8 = ypool.tile([P, NT, S], fp8)
                        Vt = vresp.tile([P, NKB, D], fp8)
                        for kb in range(NKB):
                            nc.gpsimd.dma_start(
                                Vt[:, kb, :],
                                vgN[kb * P:(kb + 1) * P, :])

                        head_state = {}

                        def finalize_head(h, Zp_h, Yp_h):
                            urow = rows.tile([1, S], fp32, name=f"u_{h}",
                                             tag="urow", bufs=2)
                            nc.vector.reciprocal_approx_fast(urow[:],
                                                             Zp_h[:])
                            ubf = rows.tile([1, S], bf16, name=f"ubf_{h}",
                                            tag="ubf", bufs=2)
                            nc.vector.tensor_copy(ubf[:], urow[:])
                            Us = apool.tile([P, S], bf16, tag="Us",
                                            name=f"Us_{h}", bufs=2)
                            nc.gpsimd.partition_broadcast(Us[:], ubf[:])
                            nc.vector.scalar_tensor_tensor(
                                y8[:, h, :], Yp_h[:], 1.0, Us[:],
                                op0=ALU.mult, op1=ALU.mult)

                        for hh in range(H):
                            # Kh loads ride the sync queue: the gpsimd
                            # queue is blocked until the V AllGather
                            # finishes, but kg is ready much earlier
                            Kh = kpool.tile([P, NKB * P], fp8, tag="Kh",
                                            name=f"Kh_{hh}")
                            for s_ in range(GROUP):
                                nc.sync.dma_start(
                                    Kh[:, s_ * S:(s_ + 1) * S],
                                    kg[s_ * D + hh * P:
                                       s_ * D + (hh + 1) * P, :])
                            Zp = atps.tile([1, S], fp32, name=f"Z_{hh}",
                                           tag="Zp", bufs=2)
                            Yp = atps.tile([P, S], fp32, name=f"Y_{hh}",
                                           tag="Yp", bufs=2)
                            ats = {}

                            def do_pair(c, hh=hh, Kh=Kh, ats=ats):
                                Lp2 = atps.tile([P, 2, S], fp32,
                                                name=f"L_{hh}_{c}",
                                                tag="logits", bufs=2)
                                for i in range(2):
                                    kb = 2 * c + i
                                    nc.tensor.matmul(
                                        Lp2[:, i, :],
                                        Kh[:, kb * P:(kb + 1) * P],
                                        q8[:, hh, :])
                                At2 = apool.tile(
                                    [P, 2, S], fp8, tag="At",
                                    name=f"At_{hh}_{c}", bufs=4)
                                nc.scalar.activation(
                                    At2[:], Lp2[:],
                                    AF.Exp, scale=ISCALE / (SA * SA))
                                ats[c] = At2

                            do_pair(0)
                            do_pair(1)
                            for c in range(NKB // 2):
                                if c + 2 < NKB // 2:
                                    do_pair(c + 2)
                                At2 = ats[c]
                                nc.tensor.matmul(Zp[:],
                                                 ones_z[:, :, 0:1],
                                                 At2[:],
                                                 start=(c == 0),
                                                 stop=(c == NKB // 2 - 1),
                                                 perf_mode=DR)
                                nc.tensor.matmul(
                                    Yp[:],
                                    Vt[:, 2 * c:2 * c + 2,
                                       hh * P:(hh + 1) * P],
                                    At2[:],
                                    start=(c == 0),
                                    stop=(c == NKB // 2 - 1),
                                    perf_mode=DR)
                                if c == 2 and hh > 0:
                                    finalize_head(hh - 1,
                                                  *head_state[hh - 1])
                            head_state[hh] = (Zp, Yp)
                        finalize_head(H - 1, *head_state[H - 1])

                  # ---- phase C: o-proj + residual (fp8, y at scale 32);
                  # LN2 stats accumulate per-j right here so phase D's
                  # LayerNorm only has the row chain + affine left ----
                  x2 = x2p.tile([P, NT, S], bf16, name="x2")
                  ln2S1s = rows.tile([1, S], fp32, name="ln2_S1s")
                  ln2S2s = rows.tile([1, S], fp32, name="ln2_S2s")
                  with (
                      tc.tile_pool(name="wproj2", bufs=1) as wpool2,
                      tc.tile_pool(name="pj2ps", bufs=1,
                                   space="PSUM") as pj2ps,
                  ):
                      ln2S1 = pj2ps.tile([1, S], fp32, name="ln2_S1")
                      ln2S2 = pj2ps.tile([1, S], fp32, name="ln2_S2")
                      def o_epi(j, main, sig):
                          tmp = wpool2.tile([P, S], fp32, tag="o_tmp",
                                            name=f"o_tmp_{j}", bufs=3)
                          nc.vector.scalar_tensor_tensor(
                              tmp[:], main[:], bias["bo"][:, j:j + 1],
                              sig[:], op0=ALU.add, op1=ALU.mult)
                          nc.vector.scalar_tensor_tensor(
                              x2[:, j, :], tmp[:], 1.0 / (SA * WS),
                              xt[:, j, :], op0=ALU.mult, op1=ALU.add)
                          ln_stats(x2[:, j, :], j, ln2S1, ln2S2, wpool2,
                                   "ln2")

                      proj_gated_f8(y8, NT, NT, "Wo", "Wgo", "bgo",
                                    1.0 / (SA * WS), wpool2, pj2ps, o_epi)
                      nc.vector.tensor_copy(ln2S1s[:], ln2S1[:])
                      nc.vector.tensor_copy(ln2S2s[:], ln2S2[:])

              # ---- phase D: LN2 + MLP (mains bf16, gates fp8); one psum
              # pool spans both MLP stages so the ring never drains ----
              with (
                  tc.tile_pool(name="midp", bufs=1) as midp,
                  tc.tile_pool(name="mlpps", bufs=1, space="PSUM") as mlpps,
              ):
                  mid = midp.tile([P, NF, S], bf16)
                  mid8 = midp.tile([P, NF, S], fp8)
                  with tc.tile_pool(name="h2p", bufs=1) as h2p:
                      with (
                          tc.tile_pool(name="ln2tmp", bufs=1) as ln2tmp,
                          tc.tile_pool(name="ln2ps", bufs=1,
                                       space="PSUM") as ln2ps,
                      ):
                          h2 = ln_T(x2, "g2", "bt2", h2p, ln2tmp, ln2ps,
                                    "ln2", stats=(ln2S1s, ln2S2s))
                      with tc.tile_pool(name="wmlp1", bufs=1) as wm1:
                          # main (bf16) accumulation + fp8 gate; epilogue:
                          # t1 = (z + b) * sig; e = erf(t1 / sqrt2);
                          # mid = (e + 1) * t1 = 2*gelu (0.5 folded into
                          # W_out/Wg_out host-side); mid8 = fp8 copy.
                          for j in range(NF):
                              main = mlpps.tile([P, S], fp32,
                                               name=f"Win_m{j}",
                                               tag="pj_main", bufs=2)
                              gate = mlpps.tile([P, S], fp32,
                                               name=f"Win_g{j}",
                                               tag="pj_gate", bufs=2)
                              wt = wm1.tile([P, NT // 2, 2, P], fp8,
                                            tag="wmain",
                                            name=f"w_Win_{j}", bufs=3)
                              nc.sync.dma_start(wt[:], wd["Win"].ap()[j])
                              for c in range(NT // 2):
                                  nc.tensor.matmul(
                                      main[:], wt[:, c, :, :],
                                      h2[:, 2 * c:2 * c + 2, :],
                                      start=(c == 0),
                                      stop=(c == NT // 2 - 1),
                                      perf_mode=DR)
                              wg = wm1.tile([P, NT // 2, 2, P], fp8,
                                            tag="wgate",
                                            name=f"w_Wgin_{j}", bufs=3)
                              nc.sync.dma_start(wg[:], wd["Wgin"].ap()[j])
                              for c in range(NT // 2):
                                  nc.tensor.matmul(
                                      gate[:], wg[:, c, :, :],
                                      h2[:, 2 * c:2 * c + 2, :],
                                      start=(c == 0),
                                      stop=(c == NT // 2 - 1),
                                      perf_mode=DR)
                              sig = wm1.tile([P, S], bf16, tag="sig",
                                             name=f"sig_Win_{j}", bufs=3)
                              nc.scalar.activation(
                                  sig[:], gate[:], AF.Sigmoid,
                                  bias=bias["bgin"][:, j:j + 1],
                                  scale=1.0 / WS)
                              t1 = wm1.tile([P, S], bf16, tag="mid_t1",
                                            name=f"t1_{j}", bufs=3)
                              nc.vector.scalar_tensor_tensor(
                                  t1[:], main[:],
                                  bias["bin"][:, j:j + 1], sig[:],
                                  op0=ALU.add, op1=ALU.mult)
                              er = wm1.tile([P, S], bf16, tag="mid_er",
                                            name=f"er_{j}", bufs=3)
                              nc.scalar.activation(er[:], t1[:], AF.Erf,
                                                   scale=RSQRT2 / SA)
                              e1 = wm1.tile([P, S], bf16, tag="mid_e1",
                                            name=f"e1_{j}", bufs=3)
                              nc.vector.tensor_scalar_add(e1[:], er[:],
                                                          1.0)
                              nc.vector.scalar_tensor_tensor(
                                  mid[:, j, :], t1[:], 1.0 / SA, e1[:],
                                  op0=ALU.mult, op1=ALU.mult)
                              nc.vector.tensor_scalar_mul(
                                  mid8[:, j, :], mid[:, j, :], SA / 2.0)

                  with tc.tile_pool(name="wmlp2", bufs=1) as wm2:
                      # W_out main bf16 (x0.5 host), gate fp8 (x32 host)
                      for j in range(NT):
                          main = mlpps.tile([P, S], fp32,
                                           name=f"Wout_m{j}",
                                           tag="pj_main", bufs=2)
                          gate = mlpps.tile([P, S], fp32,
                                           name=f"Wout_g{j}",
                                           tag="pj_gate", bufs=2)
                          for ci in range(2):
                              wt = wm2.tile([P, 32, P], bf16, tag="wmain",
                                            name=f"w_Wout_{j}_{ci}",
                                            bufs=2)
                              nc.sync.dma_start(
                                  wt[:],
                                  wd["Wout"].ap()[j, :,
                                                  32 * ci:32 * (ci + 1),
                                                  :])
                              for tt_ in range(32):
                                  t = 32 * ci + tt_
                                  nc.tensor.matmul(main[:], wt[:, tt_, :],
                                                   mid[:, t, :],
                                                   start=(t == 0),
                                                   stop=(t == NF - 1))
                          for ci in range(2):
                              wg = wm2.tile([P, 16, 2, P], fp8,
                                            tag="wgate",
                                            name=f"w_Wgout_{j}_{ci}",
                                            bufs=2)
                              nc.sync.dma_start(
                                  wg[:],
                                  wd["Wgout"].ap()[j, :,
                                                   16 * ci:16 * (ci + 1),
                                                   :, :])
                              for cc in range(16):
                                  c = 16 * ci + cc
                                  nc.tensor.matmul(
                                      gate[:], wg[:, cc, :, :],
                                      mid8[:, 2 * c:2 * c + 2, :],
                                      start=(c == 0),
                                      stop=(c == NF // 2 - 1),
                                      perf_mode=DR)
                          sig = wm2.tile([P, S], bf16, tag="sig",
                                         name=f"sig_Wout_{j}", bufs=3)
                          nc.scalar.activation(
                              sig[:], gate[:], AF.Sigmoid,
                              bias=bias["bgout"][:, j:j + 1],
                              scale=1.0 / WS)
                          tmp = wm2.tile([P, S], fp32, tag="out_tmp",
                                         name=f"out_tmp_{j}", bufs=3)
                          nc.vector.scalar_tensor_tensor(
                              tmp[:], main[:], bias["bout"][:, j:j + 1],
                              sig[:], op0=ALU.add, op1=ALU.mult)
                          outf = wm2.tile([P, S], fp32, tag="out_f",
                                          name=f"out_f_{j}", bufs=3)
                          nc.vector.tensor_tensor(outf[:], tmp[:],
                                                  x2[:, j, :], op=ALU.add)
                          nc.sync.dma_start(
                              out_d.ap()[j * P:(j + 1) * P, :], outf[:])

    nc.compile()
    return nc


def _prep_shared_inputs(inputs):
    m = {}
    for nm, w, sc in (("Wq", "W_q", SA), ("Wgq", "Wg_q", WS),
                      ("Wk", "W_k", SA), ("Wgk", "Wg_k", WS),
                      ("Wo", "W_o", WS), ("Wgo", "Wg_o", WS)):
        m[nm] = _w_tiled_f8(np.asarray(inputs[w]), sc)
    m["Wgin"] = _w_tiled_f8(np.asarray(inputs["Wg_in"]), WS)
    # mid8 carries 32*gelu, so the gate weights need only x2 to reach the
    # uniform 64x gate-psum scale; W_out main keeps the 0.5 gelu fold
    m["Wgout"] = _w_tiled_f8(np.asarray(inputs["Wg_out"]), WS / SA)
    m["Win"] = _w_tiled_f8(np.asarray(inputs["W_in"]), SA)
    m["Wout"] = _w_tiled_bf(np.asarray(inputs["W_out"]), 0.5)
    m["WvT"] = _wT_pairs_f8(np.asarray(inputs["W_v"]), SA)
    m["WgvT"] = _wT_pairs_f8(np.asarray(inputs["Wg_v"]), WS)
    m["bvrow"] = (np.asarray(inputs["b_v"]) * SA).astype(_BF).reshape(1, D)
    m["bgvrow"] = (np.asarray(inputs["bg_v"]) * WS).astype(_BF).reshape(1, D)
    for nm, bn, sc in (("bq", "b_q", SA), ("bgq", "bg_q", 1.0),
                       ("bk", "b_k", SA), ("bgk", "bg_k", 1.0),
                       ("bo", "b_o", SA * WS), ("bgo", "bg_o", 1.0),
                       ("bin", "b_in", SA), ("bgin", "bg_in", 1.0),
                       ("bout", "b_out", 1.0), ("bgout", "bg_out", 1.0),
                       ("g1", "ln1_g", 1.0), ("bt1", "ln1_b", 1.0),
                       ("g2", "ln2_g", 1.0), ("bt2", "ln2_b", 1.0)):
        m[nm] = _b_cols(np.asarray(inputs[bn]), sc)
    return m


def _install_trace_shim():
    """Provide antenv.axon_hooks (NTFF profiling) if the image lacks it."""
    import contextlib
    import ctypes
    import types

    try:
        import antenv.axon_hooks  # noqa: F401
        return
    except ImportError:
        pass
    try:
        import antenv
    except ImportError:
        return
    so_path = "/opt/axon/libaxon_pjrt.so"
    try:
        lib = ctypes.CDLL(so_path)
    except OSError:
        return
    if not hasattr(lib, "axon_start_nrt_profile"):
        return
    lib.axon_start_nrt_profile.argtypes = [ctypes.POINTER(ctypes.c_int64),
                                           ctypes.c_size_t]
    lib.axon_start_nrt_profile.restype = ctypes.c_int64
    lib.axon_stop_nrt_profile.argtypes = [ctypes.c_char_p]
    lib.axon_stop_nrt_profile.restype = ctypes.c_int64

    @contextlib.contextmanager
    def hook(output_dir, device_ids):
        import jax

        jax.devices()
        if device_ids:
            ids = (ctypes.c_int64 * len(device_ids))(*device_ids)
            rc = lib.axon_start_nrt_profile(ids, len(device_ids))
        else:
            rc = lib.axon_start_nrt_profile(None, 0)
        if rc != 0:
            raise RuntimeError(f"axon_start_nrt_profile rc={rc}")
        try:
            yield
        finally:
            n = lib.axon_stop_nrt_profile(str(output_dir).encode())
            print(f"profile: {n} ntff file(s) in {output_dir}",
                  file=sys.stderr)

    mod = types.ModuleType("antenv.axon_hooks")
    mod.get_axon_ntff_profile_hook = lambda: hook
    mod.set_axon_ntff_profile_hook = lambda h: None
    sys.modules["antenv.axon_hooks"] = mod
    antenv.axon_hooks = mod


LAST_RESULTS = None


def kernel(_trace=False, **inputs):
    global _COMPILED, LAST_RESULTS
    from concourse import bass_utils

    if _trace:
        _install_trace_shim()

    if _COMPILED is None:
        _COMPILED = _build()
    nc = _COMPILED

    shared = _prep_shared_inputs(inputs)
    x = np.asarray(inputs["x"], dtype=np.float32)  # [B, T, D]
    in_maps = []
    for c in range(N_CORES):
        g, s = divmod(c, GROUP)
        xT_c = np.ascontiguousarray(x[g, s * S:(s + 1) * S, :].T.astype(_BF))
        m = dict(shared)
        m["xT"] = xT_c
        in_maps.append(m)

    LAST_RESULTS = bass_utils.run_bass_kernel_spmd(
        nc, in_maps, core_ids=list(range(N_CORES)), trace=_trace)

    out = np.empty((B, T, D), dtype=np.float32)
    for c in range(N_CORES):
        g, s = divmod(c, GROUP)
        out[g, s * S:(s + 1) * S, :] = LAST_RESULTS.results[c]["outT"].T
    return out


# revision 57
# speedup vs baseline: 1.1185x; 1.0181x over previous
"""Trainium2 Bass kernel for nn_DGEBlock (dense transformer block with
MoE-gated linears), distributed over 8 NeuronCores.

Sharding: data-parallel over batch (2 groups of 4 cores) x sequence-parallel
over tokens within each batch (512 tokens per core). Weights replicated.
Activations live feature-major ("T-layout": [d, tok]) in SBUF; V is
projected token-major (N-layout) so attention PV needs no transposes.

Precision plan (tolerance 2e-2): fp8-e4m3 DoubleRow matmuls (2 k-tiles per
instruction, ~2x PE throughput) for everything except the W_out main, which
stays bf16 — its quantization error passes unsquashed into the residual and
is the single largest error term (sim: +1.7e-2 alone). Measured HW error
for this mix: 1.69e-2. Weights are pre-scaled on host (x64 gates, x32
q/k/v/W_in mains) so no fp8 value exceeds TRN e4m3's +-240 -> Inf; epilogue
scales fold everything back out. GELU is computed via Erf (same ACT table
set as Sigmoid -> no table thrash; the 0.5 folds into W_out host-side;
mid8 carries 32*gelu so the fp8 range is safe).

Scheduling: K/V are gathered as fp8. A collective's DMA descriptors
head-of-line block every DMA queued behind them until all peers deliver
(~30-55us of skew), so the V and first-half-Q weights are fully prefetched
into SBUF on the gpsimd queue before the K gather can trigger, and the
attention Kh loads ride the sync queue (the gpsimd queue is parked on the
V gather). LN2 statistics accumulate inside the o-projection epilogues;
paired logit-EXPs ([128,2,512] PSUM reads) halve ACT instruction count in
attention; softmax 1/Z runs via reciprocal_approx_fast + a gpsimd
partition_broadcast instead of a PE broadcast matmul.
"""

import sys

for _p in ("/opt/trn_rl_repo",):
    if _p not in sys.path:
        sys.path.append(_p)

import numpy as np
import ml_dtypes

# ---------------------------------------------------------------- constants
B = 2
T = 2048
D = 2048
H = 16
HD = 128
FF = 4 * D  # 8192
EPS = 1e-5

N_CORES = 8
GROUP = 4  # cores per batch group (sequence-parallel degree)
S = T // GROUP  # tokens per core = 512
P = 128
NT = D // P  # 16 feature tiles
NF = FF // P  # 64 hidden tiles
NKB = T // P  # 16 key blocks per batch
ISCALE = 1.0 / float(np.sqrt(HD))

WS = 64.0  # fp8 weight pre-scale (gate paths)
SA = 32.0  # q/k/v main-path weight pre-scale: keeps the fp8 activations
           # (32*k etc., |.| <~ 160) inside TRN e4m3's +-240 range
RSQRT2 = 1.0 / float(np.sqrt(2.0))

RG = [[0, 1, 2, 3], [4, 5, 6, 7]]

_BF = ml_dtypes.bfloat16
_F8 = ml_dtypes.float8_e4m3  # IEEE e4m3, max 240 == TRN FP8_EXP4

_COMPILED = None


# ------------------------------------------------------------- host prep
def _w_tiled_bf(W, scale=1.0):
    """W [dout, din] -> [nj, 128, nt, 128] bf16: W^T tile (t, j) layout;
    [j, p, t, jc] == W[j*128+jc, t*128+p]."""
    dout, din = W.shape
    nj, nt = dout // P, din // P
    return np.ascontiguousarray(
        (W.reshape(nj, P, nt, P) * scale).transpose(0, 3, 2, 1).astype(_BF)
    )


def _w_tiled_f8(W, scale=WS):
    """W [dout, din] -> [nj, 128, nt//2, 2, 128] fp8 DoubleRow pairs:
    [j, p, c, i, jc] == scale * W[j*128+jc, (2c+i)*128+p]."""
    dout, din = W.shape
    nj, nt = dout // P, din // P
    w = (W.reshape(nj, P, nt // 2, 2, P) * scale).transpose(0, 4, 2, 3, 1)
    w = np.clip(w, -240.0, 240.0)
    return np.ascontiguousarray(w.astype(_F8))


def _wT_pairs_f8(W, scale=WS):
    """W [dout, din] -> W^T pair layout [din//256, 128, 2, dout] fp8:
    [c, p, i, col] == scale * W[col, (2c+i)*128+p] (N-layout rhs)."""
    dout, din = W.shape
    w = (W.T.reshape(din // 256, 2, P, dout) * scale).transpose(0, 2, 1, 3)
    w = np.clip(w, -240.0, 240.0)
    return np.ascontiguousarray(w.astype(_F8))


def _b_cols(b, scale=1.0):
    """b [dout] -> [128, nj] fp32: column j holds b[j*128:(j+1)*128]."""
    nj = b.shape[0] // P
    return np.ascontiguousarray((b.reshape(nj, P) * scale).T.astype(np.float32))


# ------------------------------------------------------------- device build
def _build():
    from concourse import bacc, tile, mybir

    fp32 = mybir.dt.float32
    bf16 = mybir.dt.bfloat16
    fp8 = mybir.dt.float8e4
    AF = mybir.ActivationFunctionType
    ALU = mybir.AluOpType
    DR = mybir.MatmulPerfMode.DoubleRow

    nc = bacc.Bacc("TRN2", target_bir_lowering=False, debug=False,
                   num_devices=N_CORES)

    # ---- I/O tensors
    xT_d = nc.dram_tensor("xT", [D, S], bf16, kind="ExternalInput")
    wd = {}
    # fp8 DoubleRow pair weights (scale x64): qkvo mains + all gates
    for nm in ("Wq", "Wgq", "Wk", "Wgk", "Wo", "Wgo"):
        wd[nm] = nc.dram_tensor(nm, [NT, P, NT // 2, 2, P], fp8,
                                kind="ExternalInput")
    wd["Wgin"] = nc.dram_tensor("Wgin", [NF, P, NT // 2, 2, P], fp8,
                                kind="ExternalInput")
    wd["Wgout"] = nc.dram_tensor("Wgout", [NT, P, NF // 2, 2, P], fp8,
                                 kind="ExternalInput")
    wd["Win"] = nc.dram_tensor("Win", [NF, P, NT // 2, 2, P], fp8,
                               kind="ExternalInput")
    # bf16 main for the MLP output projection (accuracy-critical)
    wd["Wout"] = nc.dram_tensor("Wout", [NT, P, NF, P], bf16,
                                kind="ExternalInput")
    # V projection runs in N-layout: W^T fp8 pair layout + bias rows (x64)
    wd["WvT"] = nc.dram_tensor("WvT", [NT // 2, P, 2, D], fp8,
                               kind="ExternalInput")
    wd["WgvT"] = nc.dram_tensor("WgvT", [NT // 2, P, 2, D], fp8,
                                kind="ExternalInput")
    bvrow_d = nc.dram_tensor("bvrow", [1, D], bf16, kind="ExternalInput")
    bgvrow_d = nc.dram_tensor("bgvrow", [1, D], bf16, kind="ExternalInput")
    bd = {}
    for nm in ("bq", "bgq", "bk", "bgk", "bo", "bgo",
               "bout", "bgout", "g1", "bt1", "g2", "bt2"):
        bd[nm] = nc.dram_tensor(nm, [P, NT], fp32, kind="ExternalInput")
    for nm in ("bin", "bgin"):
        bd[nm] = nc.dram_tensor(nm, [P, NF], fp32, kind="ExternalInput")
    out_d = nc.dram_tensor("outT", [D, S], fp32, kind="ExternalOutput")

    with tile.TileContext(nc) as tc:
        with (
            tc.tile_pool(name="const", bufs=1) as constp,
            tc.tile_pool(name="bias", bufs=1) as biasp,
            tc.tile_pool(name="rows", bufs=1) as rows,
            tc.tile_pool(name="dram", bufs=1, space="DRAM") as dramp,
        ):
            ones_col = constp.tile([P, 1], bf16)
            nc.vector.memset(ones_col[:], 1.0)
            ones_row = constp.tile([1, P], bf16)
            nc.vector.memset(ones_row[:], 1.0)
            # fp8 ones for DoubleRow Z-sums; padded so the pair-dim byte
            # stride is 16 (DoubleRow AP constraint)
            ones_z = constp.tile([P, 2, 16], fp8)
            nc.vector.memset(ones_z[:], 1.0)
            eps_t = constp.tile([1, 1], fp32)
            nc.vector.memset(eps_t[:], EPS)
            # bias/row constants load via the scalar queue so the sync
            # queue starts streaming x immediately
            bvrow = constp.tile([1, D], bf16)
            nc.scalar.dma_start(bvrow[:], bvrow_d.ap())
            bgvrow = constp.tile([1, D], bf16)
            nc.scalar.dma_start(bgvrow[:], bgvrow_d.ap())

            bias = {}
            for nm in bd:
                ncols = NF if nm in ("bin", "bgin") else NT
                btile = biasp.tile([P, ncols], fp32, name=f"bias_{nm}")
                nc.scalar.dma_start(btile[:], bd[nm].ap())
                bias[nm] = btile

            # ---------- helpers ----------
            def ln_stats(src_j, j, S1, S2, sqpool, name):
                """One tile's contribution to LN stats (callable per-j from
                an earlier phase's epilogue to hide the latency). Squares
                alternate ACT/DVE so neither engine's serial chain binds."""
                sq = sqpool.tile([P, S], bf16, name=f"{name}_sq{j}",
                                 tag=f"{name}_sq", bufs=6)
                if j % 2 == 0:
                    nc.scalar.activation(sq[:], src_j, AF.Square)
                else:
                    nc.vector.tensor_tensor(sq[:], src_j, src_j,
                                            op=ALU.mult)
                nc.tensor.matmul(S1[:], ones_col[:], src_j,
                                 start=(j == 0), stop=(j == NT - 1))
                nc.tensor.matmul(S2[:], ones_col[:], sq[:],
                                 start=(j == 0), stop=(j == NT - 1))

            def ln_T(src, gname, bname, hpool, tmpool, psln, name,
                     stats=None):
                """LayerNorm over features of a T-layout activation.

                src: SBUF [128, NT, S] bf16 -> [128, NT, S] (dtype per
                hpool_dtype). Stats via ones-matmuls (or passed in via
                `stats` if accumulated earlier); per-token scale/shift rows
                broadcast via rank-1 matmuls; bf16 DVE affine (2x rate).
                """
                if stats is None:
                    S1 = psln.tile([1, S], fp32, name=f"{name}_S1",
                                   tag="ln_S1")
                    S2 = psln.tile([1, S], fp32, name=f"{name}_S2",
                                   tag="ln_S2")
                    for t in range(NT):
                        ln_stats(src[:, t, :], t, S1, S2, tmpool, name)
                else:
                    S1, S2 = stats

                def row(nm, dt=fp32):
                    return rows.tile([1, S], dt, name=f"{name}_{nm}",
                                     tag=f"ln_{nm}")

                mean = row("mean")
                nc.vector.tensor_scalar_mul(mean[:], S1[:], 1.0 / D)
                m2 = row("m2")
                nc.vector.tensor_scalar_mul(m2[:], S2[:], 1.0 / D)
                msq = row("msq")
                nc.vector.tensor_tensor(msq[:], mean[:], mean[:],
                                        op=ALU.mult)
                var = row("var")
                nc.vector.tensor_tensor(var[:], m2[:], msq[:],
                                        op=ALU.subtract)
                std = row("std")
                nc.scalar.activation(std[:], var[:], AF.Sqrt,
                                     bias=eps_t[:])
                rstd = row("rstd")
                nc.vector.reciprocal_approx_fast(rstd[:], std[:])
                rstd_bf = row("rstdbf", bf16)
                nc.vector.tensor_copy(rstd_bf[:], rstd[:])
                mr_bf = row("mrbf", bf16)
                nc.vector.tensor_tensor(mr_bf[:], mean[:], rstd[:],
                                        op=ALU.mult)
                Ab_p = psln.tile([P, S], fp32, name=f"{name}_Abp",
                                 tag="ln_Abp")
                nc.tensor.matmul(Ab_p[:], ones_row[:], rstd_bf[:])
                Bb_p = psln.tile([P, S], fp32, name=f"{name}_Bbp",
                                 tag="ln_Bbp")
                nc.tensor.matmul(Bb_p[:], ones_row[:], mr_bf[:])
                Ab = tmpool.tile([P, S], bf16, name=f"{name}_Ab")
                nc.vector.tensor_copy(Ab[:], Ab_p[:])
                Bb = tmpool.tile([P, S], bf16, name=f"{name}_Bb")
                nc.vector.tensor_copy(Bb[:], Bb_p[:])
                h = hpool.tile([P, NT, S], hpool_dtype[name],
                               name=f"{name}_h")
                for t in range(NT):
                    tmp = tmpool.tile([P, S], bf16, name=f"{name}_t0_{t}",
                                      tag="ln_t0", bufs=6)
                    nc.vector.tensor_tensor(tmp[:], src[:, t, :], Ab[:],
                                            op=ALU.mult)
                    tmp2 = tmpool.tile([P, S], bf16, name=f"{name}_t1_{t}",
                                       tag="ln_t1", bufs=6)
                    nc.vector.tensor_tensor(tmp2[:], tmp[:], Bb[:],
                                            op=ALU.subtract)
                    nc.scalar.activation(h[:, t, :], tmp2[:], AF.Identity,
                                         bias=bias[bname][:, t:t + 1],
                                         scale=bias[gname][:, t:t + 1])
                return h

            hpool_dtype = {"ln1": fp8, "ln2": fp8}

            def proj_gated_f8(src, nt, nj, wname, wgname, bgname, sig_scale,
                              wpool, pspool, epilogue, cchunk=None, wbufs=3):
                """Gated projection, T-layout, fp8 DoubleRow: for each output
                tile j, main/gate = sum_c W8(c,j).T @ src[:, 2c:2c+2, :],
                then epilogue(j, main_psum, sig_sbuf). src fp8 [P, nt, S]."""
                npair = nt // 2
                if cchunk is None:
                    cchunk = npair
                nchunk = npair // cchunk
                for j in range(nj):
                    main = pspool.tile([P, S], fp32, name=f"{wname}_m{j}",
                                       tag="pj_main", bufs=2)
                    gate = pspool.tile([P, S], fp32, name=f"{wname}_g{j}",
                                       tag="pj_gate", bufs=2)
                    for ci in range(nchunk):
                        wt = wpool.tile([P, cchunk, 2, P], fp8, tag="wmain",
                                        name=f"w_{wname}_{j}_{ci}",
                                        bufs=wbufs)
                        nc.sync.dma_start(
                            wt[:],
                            wd[wname].ap()[j, :,
                                           ci * cchunk:(ci + 1) * cchunk,
                                           :, :])
                        for cc in range(cchunk):
                            c = ci * cchunk + cc
                            nc.tensor.matmul(main[:], wt[:, cc, :, :],
                                             src[:, 2 * c:2 * c + 2, :],
                                             start=(c == 0),
                                             stop=(c == npair - 1),
                                             perf_mode=DR)
                    for ci in range(nchunk):
                        wg = wpool.tile([P, cchunk, 2, P], fp8, tag="wgate",
                                        name=f"w_{wgname}_{j}_{ci}",
                                        bufs=wbufs)
                        nc.sync.dma_start(
                            wg[:],
                            wd[wgname].ap()[j, :,
                                            ci * cchunk:(ci + 1) * cchunk,
                                            :, :])
                        for cc in range(cchunk):
                            c = ci * cchunk + cc
                            nc.tensor.matmul(gate[:], wg[:, cc, :, :],
                                             src[:, 2 * c:2 * c + 2, :],
                                             start=(c == 0),
                                             stop=(c == npair - 1),
                                             perf_mode=DR)
                    sig = wpool.tile([P, S], bf16, tag="sig",
                                     name=f"sig_{wname}_{j}", bufs=3)
                    nc.scalar.activation(sig[:], gate[:], AF.Sigmoid,
                                         bias=bias[bgname][:, j:j + 1],
                                         scale=sig_scale)
                    epilogue(j, main, sig)

            # x2 outlives phases A-C (used by LN2 + MLP residual)
            with tc.tile_pool(name="x2p", bufs=1) as x2p:
              with tc.tile_pool(name="xt", bufs=1) as xtp:
                xt = xtp.tile([P, NT, S], bf16)
                xT_v = xT_d.ap().rearrange("(t p) s -> t p s", p=P)
                for t in range(NT):
                    # split across two queues so two DMA rings fetch x in
                    # parallel (the gpsimd queue is idle this early)
                    eng = nc.sync if t % 2 == 0 else nc.gpsimd
                    eng.dma_start(xt[:, t, :], xT_v[t])

                vN_bounce = dramp.tile([S, D], fp8)
                k_bounce = dramp.tile([D, S], fp8)
                vgN = dramp.tile([GROUP * S, D], fp8)
                kg = dramp.tile([GROUP * D, S], fp8)

                with tc.tile_pool(name="yp", bufs=1) as ypool:
                  with tc.tile_pool(name="qp", bufs=1) as qpool:
                    q8 = qpool.tile([P, NT, S], fp8)

                    with tc.tile_pool(name="hq", bufs=1) as hqp:
                        with (
                            tc.tile_pool(name="ln1tmp", bufs=1) as ln1tmp,
                            tc.tile_pool(name="ln1ps", bufs=1,
                                         space="PSUM") as ln1ps,
                        ):
                            h1 = ln_T(xt, "g1", "bt1", hqp, ln1tmp, ln1ps,
                                      "ln1")

                        # Q weights are fully preloaded on the (otherwise
                        # idle) GpSimd DMA queue, spread through the K and V
                        # phases. The collectives HOL-block every DMA queued
                        # behind them, so Q must not need DMA after the
                        # gathers trigger.
                        # prefetch only the first half of Q's weights: the
                        # later j's compute after the gather window closes,
                        # so they can stream just-in-time on the sync queue
                        NQPRE = NT // 2
                        qw, qgw = {}, {}
                        for j in range(NQPRE):
                            qw[j] = hqp.tile([P, NT // 2, 2, P], fp8,
                                             tag="wqpre", bufs=NQPRE,
                                             name=f"wq_{j}")
                            qgw[j] = hqp.tile([P, NT // 2, 2, P], fp8,
                                              tag="wgqpre", bufs=NQPRE,
                                              name=f"wgq_{j}")

                        # V weight tiles: prefetched on the gpsimd queue
                        # during the K phase (V compute consumes them first;
                        # Q's prefetch follows during the V phase)
                        NP = NT // 2  # 8 contraction pairs
                        vw, vgw = {}, {}
                        for n in range(4):
                            for c in range(NP):
                                vw[n, c] = hqp.tile(
                                    [P, 2, S], fp8, tag="wv", bufs=32,
                                    name=f"wv_{n}_{c}")
                                vgw[n, c] = hqp.tile(
                                    [P, 2, S], fp8, tag="wgv", bufs=32,
                                    name=f"wgv_{n}_{c}")

                        def vw_prefetch_step(i):
                            n, c = divmod(i // 2, NP)
                            t = vgw[n, c] if i % 2 else vw[n, c]
                            w = wd["WgvT"] if i % 2 else wd["WvT"]
                            nc.gpsimd.dma_start(
                                t[:], w.ap()[c, :, :, n * S:(n + 1) * S])

                        # ---- K projection (T-layout); its AllGather
                        # triggers as soon as k_bounce is complete, which is
                        # why V/Q weights must already be in flight/SBUF ----
                        with (
                            tc.tile_pool(name="wproj", bufs=1) as wpool,
                            tc.tile_pool(name="pjps", bufs=1,
                                         space="PSUM") as pjps,
                        ):
                            def k_epi(j, main, sig):
                                kv = wpool.tile([P, S], fp8, tag="kv_out",
                                                name=f"kv_k_{j}", bufs=8)
                                nc.vector.scalar_tensor_tensor(
                                    kv[:], main[:], bias["bk"][:, j:j + 1],
                                    sig[:], op0=ALU.add, op1=ALU.mult)
                                nc.scalar.dma_start(
                                    k_bounce[j * P:(j + 1) * P, :], kv[:])
                                for i in range(4 * j, 4 * j + 4):
                                    vw_prefetch_step(i)

                            proj_gated_f8(h1, NT, NT, "Wk", "Wgk", "bgk",
                                          1.0 / WS, wpool, pjps, k_epi)

                        # ---- V projection, N-layout, fp8 DoubleRow.
                        # All V (and Q) weights are fully prefetched on the
                        # gpsimd queue before the K-gather can trigger: once
                        # a collective's DMA descriptors are on the rings,
                        # every later-queued DMA waits for peer delivery
                        # (~30-55us), so V/Q must not need loads then. ----
                        with (
                            tc.tile_pool(name="wv", bufs=1) as wvp,
                            tc.tile_pool(name="vps", bufs=1,
                                         space="PSUM") as vps,
                        ):
                            # Q weights (first half) prefetch during V
                            for j in range(NQPRE):
                                nc.gpsimd.dma_start(qw[j][:],
                                                    wd["Wq"].ap()[j])
                                nc.gpsimd.dma_start(qgw[j][:],
                                                    wd["Wgq"].ap()[j])
                            for n in range(4):
                                vmain = [vps.tile([P, S], fp32,
                                                  tag="v_main", bufs=4,
                                                  name=f"vm_{n}_{m}")
                                         for m in range(4)]
                                vgate = [vps.tile([P, S], fp32,
                                                  tag="v_gate", bufs=4,
                                                  name=f"vg_{n}_{m}")
                                         for m in range(4)]
                                for c in range(NP):
                                    wvt = vw[n, c]
                                    wgvt = vgw[n, c]
                                    for m in range(4):
                                        nc.tensor.matmul(
                                            vmain[m][:],
                                            h1[:, 2 * c:2 * c + 2,
                                               m * P:(m + 1) * P],
                                            wvt[:],
                                            start=(c == 0), stop=False,
                                            perf_mode=DR)
                                    for m in range(4):
                                        nc.tensor.matmul(
                                            vgate[m][:],
                                            h1[:, 2 * c:2 * c + 2,
                                               m * P:(m + 1) * P],
                                            wgvt[:],
                                            start=(c == 0), stop=False,
                                            perf_mode=DR)
                                for m in range(4):
                                    nc.tensor.matmul(
                                        vmain[m][:], ones_row[:],
                                        bvrow[:, n * S:(n + 1) * S],
                                        start=False, stop=True,
                                        skip_group_check=True)
                                    nc.tensor.matmul(
                                        vgate[m][:], ones_row[:],
                                        bgvrow[:, n * S:(n + 1) * S],
                                        start=False, stop=True,
                                        skip_group_check=True)
                                    vsig = wvp.tile([P, S], bf16,
                                                    tag="vsig", bufs=3,
                                                    name=f"vsig_{n}_{m}")
                                    nc.scalar.activation(vsig[:],
                                                         vgate[m][:],
                                                         AF.Sigmoid,
                                                         scale=1.0 / WS)
                                    vout = wvp.tile([P, S], fp8,
                                                    tag="vout", bufs=16,
                                                    name=f"vout_{n}_{m}")
                                    nc.vector.scalar_tensor_tensor(
                                        vout[:], vmain[m][:], 1.0, vsig[:],
                                        op0=ALU.mult, op1=ALU.mult)
                                    nc.scalar.dma_start(
                                        vN_bounce[m * P:(m + 1) * P,
                                                  n * S:(n + 1) * S],
                                        vout[:])

                        nc.gpsimd.collective_compute(
                            "AllGather", ALU.bypass, ins=[k_bounce[:]],
                            outs=[kg[:]], replica_groups=RG)
                        nc.gpsimd.collective_compute(
                            "AllGather", ALU.bypass, ins=[vN_bounce[:]],
                            outs=[vgN[:]], replica_groups=RG)

                        # ---- Q projection (weights already in SBUF) ----
                        with (
                            tc.tile_pool(name="wprojq", bufs=1) as wpoolq,
                            tc.tile_pool(name="pjqps", bufs=1,
                                         space="PSUM") as pjqps,
                        ):
                            for j in range(NT):
                                main = pjqps.tile([P, S], fp32,
                                                  name=f"Wq_m{j}",
                                                  tag="pj_main", bufs=2)
                                gate = pjqps.tile([P, S], fp32,
                                                  name=f"Wq_g{j}",
                                                  tag="pj_gate", bufs=2)
                                if j < NQPRE:
                                    wt, wg = qw[j], qgw[j]
                                else:
                                    wt = wpoolq.tile([P, NT // 2, 2, P],
                                                     fp8, tag="wmain",
                                                     name=f"w_Wq_{j}",
                                                     bufs=3)
                                    nc.sync.dma_start(wt[:],
                                                      wd["Wq"].ap()[j])
                                    wg = wpoolq.tile([P, NT // 2, 2, P],
                                                     fp8, tag="wgate",
                                                     name=f"w_Wgq_{j}",
                                                     bufs=3)
                                    nc.sync.dma_start(wg[:],
                                                      wd["Wgq"].ap()[j])
                                for c in range(NT // 2):
                                    nc.tensor.matmul(
                                        main[:], wt[:, c, :, :],
                                        h1[:, 2 * c:2 * c + 2, :],
                                        start=(c == 0),
                                        stop=(c == NT // 2 - 1),
                                        perf_mode=DR)
                                for c in range(NT // 2):
                                    nc.tensor.matmul(
                                        gate[:], wg[:, c, :, :],
                                        h1[:, 2 * c:2 * c + 2, :],
                                        start=(c == 0),
                                        stop=(c == NT // 2 - 1),
                                        perf_mode=DR)
                                sig = wpoolq.tile([P, S], bf16, tag="sig",
                                                  name=f"sig_Wq_{j}",
                                                  bufs=3)
                                nc.scalar.activation(
                                    sig[:], gate[:], AF.Sigmoid,
                                    bias=bias["bgq"][:, j:j + 1],
                                    scale=1.0 / WS)
                                nc.vector.scalar_tensor_tensor(
                                    q8[:, j, :], main[:],
                                    bias["bq"][:, j:j + 1],
                                    sig[:], op0=ALU.add, op1=ALU.mult)

                    # ---- phase B: attention (all fp8) ----
                    with (
                        tc.tile_pool(name="vres", bufs=1) as vresp,
                        tc.tile_pool(name="kstream", bufs=2) as kpool,
                        tc.tile_pool(name="apool", bufs=4) as apool,
                        tc.tile_pool(name="atps", bufs=1,
                                     space="PSUM") as atps,
                    ):
                        y8 = ypool.tile([P, NT, S], fp8)
                        Vt = vresp.tile([P, NKB, D], fp8)
                        for kb in range(NKB):
                            nc.gpsimd.dma_start(
                                Vt[:, kb, :],
                                vgN[kb * P:(kb + 1) * P, :])

                        head_state = {}

                        def finalize_head(h, Zp_h, Yp_h):
                            urow = rows.tile([1, S], fp32, name=f"u_{h}",
                                             tag="urow", bufs=2)
                            nc.vector.reciprocal_approx_fast(urow[:],
                                                             Zp_h[:])
                            ubf = rows.tile([1, S], bf16, name=f"ubf_{h}",
                                            tag="ubf", bufs=2)
                            nc.vector.tensor_copy(ubf[:], urow[:])
                            Us = apool.tile([P, S], bf16, tag="Us",
                                            name=f"Us_{h}", bufs=2)
                            nc.gpsimd.partition_broadcast(Us[:], ubf[:])
                            nc.vector.scalar_tensor_tensor(
                                y8[:, h, :], Yp_h[:], 1.0, Us[:],
                                op0=ALU.mult, op1=ALU.mult)

                        for hh in range(H):
                            # Kh loads ride the sync queue: the gpsimd
                            # queue is blocked until the V AllGather
                            # finishes, but kg is ready much earlier
                            Kh = kpool.tile([P, NKB * P], fp8, tag="Kh",
                                            name=f"Kh_{hh}")
                            for s_ in range(GROUP):
                                nc.sync.dma_start(
                                    Kh[:, s_ * S:(s_ + 1) * S],
                                    kg[s_ * D + hh * P:
                                       s_ * D + (hh + 1) * P, :])
                            Zp = atps.tile([1, S], fp32, name=f"Z_{hh}",
                                           tag="Zp", bufs=2)
                            Yp = atps.tile([P, S], fp32, name=f"Y_{hh}",
                                           tag="Yp", bufs=2)
                            ats = {}

                            def do_pair(c, hh=hh, Kh=Kh, ats=ats):
                                Lp2 = atps.tile([P, 2, S], fp32,
                                                name=f"L_{hh}_{c}",
                                                tag="logits", bufs=2)
                                for i in range(2):
                                    kb = 2 * c + i
                                    nc.tensor.matmul(
                                        Lp2[:, i, :],
                                        Kh[:, kb * P:(kb + 1) * P],
                                        q8[:, hh, :])
                                At2 = apool.tile(
                                    [P, 2, S], fp8, tag="At",
                                    name=f"At_{hh}_{c}", bufs=6)
                                nc.scalar.activation(
                                    At2[:], Lp2[:],
                                    AF.Exp, scale=ISCALE / (SA * SA))
                                ats[c] = At2

                            do_pair(0)
                            do_pair(1)
                            for c in range(NKB // 2):
                                if c + 2 < NKB // 2:
                                    do_pair(c + 2)
                                At2 = ats[c]
                                nc.tensor.matmul(Zp[:],
                                                 ones_z[:, :, 0:1],
                                                 At2[:],
                                                 start=(c == 0),
                                                 stop=(c == NKB // 2 - 1),
                                                 perf_mode=DR)
                                nc.tensor.matmul(
                                    Yp[:],
                                    Vt[:, 2 * c:2 * c + 2,
                                       hh * P:(hh + 1) * P],
                                    At2[:],
                                    start=(c == 0),
                                    stop=(c == NKB // 2 - 1),
                                    perf_mode=DR)
                                if c == 2 and hh > 0:
                                    finalize_head(hh - 1,
                                                  *head_state[hh - 1])
                            head_state[hh] = (Zp, Yp)
                        finalize_head(H - 1, *head_state[H - 1])

                  # ---- phase C: o-proj + residual (fp8, y at scale 32);
                  # LN2 stats accumulate per-j right here so phase D's
                  # LayerNorm only has the row chain + affine left ----
                  x2 = x2p.tile([P, NT, S], bf16, name="x2")
                  ln2S1s = rows.tile([1, S], fp32, name="ln2_S1s")
                  ln2S2s = rows.tile([1, S], fp32, name="ln2_S2s")
                  with (
                      tc.tile_pool(name="wproj2", bufs=1) as wpool2,
                      tc.tile_pool(name="pj2ps", bufs=1,
                                   space="PSUM") as pj2ps,
                  ):
                      ln2S1 = pj2ps.tile([1, S], fp32, name="ln2_S1")
                      ln2S2 = pj2ps.tile([1, S], fp32, name="ln2_S2")
                      def o_epi(j, main, sig):
                          tmp = wpool2.tile([P, S], fp32, tag="o_tmp",
                                            name=f"o_tmp_{j}", bufs=3)
                          nc.vector.scalar_tensor_tensor(
                              tmp[:], main[:], bias["bo"][:, j:j + 1],
                              sig[:], op0=ALU.add, op1=ALU.mult)
                          nc.vector.scalar_tensor_tensor(
                              x2[:, j, :], tmp[:], 1.0 / (SA * WS),
                              xt[:, j, :], op0=ALU.mult, op1=ALU.add)
                          ln_stats(x2[:, j, :], j, ln2S1, ln2S2, wpool2,
                                   "ln2")

                      proj_gated_f8(y8, NT, NT, "Wo", "Wgo", "bgo",
                                    1.0 / (SA * WS), wpool2, pj2ps, o_epi)
                      nc.vector.tensor_copy(ln2S1s[:], ln2S1[:])
                      nc.vector.tensor_copy(ln2S2s[:], ln2S2[:])

              # ---- phase D: LN2 + MLP (mains bf16, gates fp8); one psum
              # pool spans both MLP stages so the ring never drains ----
              with (
                  tc.tile_pool(name="midp", bufs=1) as midp,
                  tc.tile_pool(name="mlpps", bufs=1, space="PSUM") as mlpps,
              ):
                  mid = midp.tile([P, NF, S], bf16)
                  mid8 = midp.tile([P, NF, S], fp8)
                  with tc.tile_pool(name="h2p", bufs=1) as h2p:
                      with (
                          tc.tile_pool(name="ln2tmp", bufs=1) as ln2tmp,
                          tc.tile_pool(name="ln2ps", bufs=1,
                                       space="PSUM") as ln2ps,
                      ):
                          h2 = ln_T(x2, "g2", "bt2", h2p, ln2tmp, ln2ps,
                                    "ln2", stats=(ln2S1s, ln2S2s))
                      with tc.tile_pool(name="wmlp1", bufs=1) as wm1:
                          # main (bf16) accumulation + fp8 gate; epilogue:
                          # t1 = (z + b) * sig; e = erf(t1 / sqrt2);
                          # mid = (e + 1) * t1 = 2*gelu (0.5 folded into
                          # W_out/Wg_out host-side); mid8 = fp8 copy.
                          for j in range(NF):
                              main = mlpps.tile([P, S], fp32,
                                               name=f"Win_m{j}",
                                               tag="pj_main", bufs=2)
                              gate = mlpps.tile([P, S], fp32,
                                               name=f"Win_g{j}",
                                               tag="pj_gate", bufs=2)
                              wt = wm1.tile([P, NT // 2, 2, P], fp8,
                                            tag="wmain",
                                            name=f"w_Win_{j}", bufs=3)
                              nc.sync.dma_start(wt[:], wd["Win"].ap()[j])
                              for c in range(NT // 2):
                                  nc.tensor.matmul(
                                      main[:], wt[:, c, :, :],
                                      h2[:, 2 * c:2 * c + 2, :],
                                      start=(c == 0),
                                      stop=(c == NT // 2 - 1),
                                      perf_mode=DR)
                              wg = wm1.tile([P, NT // 2, 2, P], fp8,
                                            tag="wgate",
                                            name=f"w_Wgin_{j}", bufs=3)
                              nc.sync.dma_start(wg[:], wd["Wgin"].ap()[j])
                              for c in range(NT // 2):
                                  nc.tensor.matmul(
                                      gate[:], wg[:, c, :, :],
                                      h2[:, 2 * c:2 * c + 2, :],
                                      start=(c == 0),
                                      stop=(c == NT // 2 - 1),
                                      perf_mode=DR)
                              sig = wm1.tile([P, S], bf16, tag="sig",
                                             name=f"sig_Win_{j}", bufs=3)
                              nc.scalar.activation(
                                  sig[:], gate[:], AF.Sigmoid,
                                  bias=bias["bgin"][:, j:j + 1],
                                  scale=1.0 / WS)
                              t1 = wm1.tile([P, S], bf16, tag="mid_t1",
                                            name=f"t1_{j}", bufs=3)
                              nc.vector.scalar_tensor_tensor(
                                  t1[:], main[:],
                                  bias["bin"][:, j:j + 1], sig[:],
                                  op0=ALU.add, op1=ALU.mult)
                              er = wm1.tile([P, S], bf16, tag="mid_er",
                                            name=f"er_{j}", bufs=3)
                              nc.scalar.activation(er[:], t1[:], AF.Erf,
                                                   scale=RSQRT2 / SA)
                              e1 = wm1.tile([P, S], bf16, tag="mid_e1",
                                            name=f"e1_{j}", bufs=3)
                              nc.vector.tensor_scalar_add(e1[:], er[:],
                                                          1.0)
                              nc.vector.scalar_tensor_tensor(
                                  mid[:, j, :], t1[:], 1.0 / SA, e1[:],
                                  op0=ALU.mult, op1=ALU.mult)
                              nc.vector.tensor_scalar_mul(
                                  mid8[:, j, :], mid[:, j, :], SA / 2.0)

                  with tc.tile_pool(name="wmlp2", bufs=1) as wm2:
                      # W_out main bf16 (x0.5 host), gate fp8 (x32 host)
                      for j in range(NT):
                          main = mlpps.tile([P, S], fp32,
                                           name=f"Wout_m{j}",
                                           tag="pj_main", bufs=2)
                          gate = mlpps.tile([P, S], fp32,
                                           name=f"Wout_g{j}",
                                           tag="pj_gate", bufs=2)
                          for ci in range(2):
                              wt = wm2.tile([P, 32, P], bf16, tag="wmain",
                                            name=f"w_Wout_{j}_{ci}",
                                            bufs=2)
                              nc.sync.dma_start(
                                  wt[:],
                                  wd["Wout"].ap()[j, :,
                                                  32 * ci:32 * (ci + 1),
                                                  :])
                              for tt_ in range(32):
                                  t = 32 * ci + tt_
                                  nc.tensor.matmul(main[:], wt[:, tt_, :],
                                                   mid[:, t, :],
                                                   start=(t == 0),
                                                   stop=(t == NF - 1))
                          for ci in range(2):
                              wg = wm2.tile([P, 16, 2, P], fp8,
                                            tag="wgate",
                                            name=f"w_Wgout_{j}_{ci}",
                                            bufs=2)
                              nc.sync.dma_start(
                                  wg[:],
                                  wd["Wgout"].ap()[j, :,
                                                   16 * ci:16 * (ci + 1),
                                                   :, :])
                              for cc in range(16):
                                  c = 16 * ci + cc
                                  nc.tensor.matmul(
                                      gate[:], wg[:, cc, :, :],
                                      mid8[:, 2 * c:2 * c + 2, :],
                                      start=(c == 0),
                                      stop=(c == NF // 2 - 1),
                                      perf_mode=DR)
                          sig = wm2.tile([P, S], bf16, tag="sig",
                                         name=f"sig_Wout_{j}", bufs=3)
                          nc.scalar.activation(
                              sig[:], gate[:], AF.Sigmoid,
                              bias=bias["bgout"][:, j:j + 1],
                              scale=1.0 / WS)
                          tmp = wm2.tile([P, S], fp32, tag="out_tmp",
                                         name=f"out_tmp_{j}", bufs=3)
                          nc.vector.scalar_tensor_tensor(
                              tmp[:], main[:], bias["bout"][:, j:j + 1],
                              sig[:], op0=ALU.add, op1=ALU.mult)
                          outf = wm2.tile([P, S], fp32, tag="out_f",
                                          name=f"out_f_{j}", bufs=3)
                          nc.vector.tensor_tensor(outf[:], tmp[:],
                                                  x2[:, j, :], op=ALU.add)
                          nc.sync.dma_start(
                              out_d.ap()[j * P:(j + 1) * P, :], outf[:])

    nc.compile()
    return nc


def _prep_shared_inputs(inputs):
    m = {}
    for nm, w, sc in (("Wq", "W_q", SA), ("Wgq", "Wg_q", WS),
                      ("Wk", "W_k", SA), ("Wgk", "Wg_k", WS),
                      ("Wo", "W_o", WS), ("Wgo", "Wg_o", WS)):
        m[nm] = _w_tiled_f8(np.asarray(inputs[w]), sc)
    m["Wgin"] = _w_tiled_f8(np.asarray(inputs["Wg_in"]), WS)
    # mid8 carries 32*gelu, so the gate weights need only x2 to reach the
    # uniform 64x gate-psum scale; W_out main keeps the 0.5 gelu fold
    m["Wgout"] = _w_tiled_f8(np.asarray(inputs["Wg_out"]), WS / SA)
    m["Win"] = _w_tiled_f8(np.asarray(inputs["W_in"]), SA)
    m["Wout"] = _w_tiled_bf(np.asarray(inputs["W_out"]), 0.5)
    m["WvT"] = _wT_pairs_f8(np.asarray(inputs["W_v"]), SA)
    m["WgvT"] = _wT_pairs_f8(np.asarray(inputs["Wg_v"]), WS)
    m["bvrow"] = (np.asarray(inputs["b_v"]) * SA).astype(_BF).reshape(1, D)
    m["bgvrow"] = (np.asarray(inputs["bg_v"]) * WS).astype(_BF).reshape(1, D)
    for nm, bn, sc in (("bq", "b_q", SA), ("bgq", "bg_q", 1.0),
                       ("bk", "b_k", SA), ("bgk", "bg_k", 1.0),
                       ("bo", "b_o", SA * WS), ("bgo", "bg_o", 1.0),
                       ("bin", "b_in", SA), ("bgin", "bg_in", 1.0),
                       ("bout", "b_out", 1.0), ("bgout", "bg_out", 1.0),
                       ("g1", "ln1_g", 1.0), ("bt1", "ln1_b", 1.0),
                       ("g2", "ln2_g", 1.0), ("bt2", "ln2_b", 1.0)):
        m[nm] = _b_cols(np.asarray(inputs[bn]), sc)
    return m


def _install_trace_shim():
    """Provide antenv.axon_hooks (NTFF profiling) if the image lacks it."""
    import contextlib
    import ctypes
    import types

    try:
        import antenv.axon_hooks  # noqa: F401
        return
    except ImportError:
        pass
    try:
        import antenv
    except ImportError:
        return
    so_path = "/opt/axon/libaxon_pjrt.so"
    try:
        lib = ctypes.CDLL(so_path)
    except OSError:
        return
    if not hasattr(lib, "axon_start_nrt_profile"):
        return
    lib.axon_start_nrt_profile.argtypes = [ctypes.POINTER(ctypes.c_int64),
                                           ctypes.c_size_t]
    lib.axon_start_nrt_profile.restype = ctypes.c_int64
    lib.axon_stop_nrt_profile.argtypes = [ctypes.c_char_p]
    lib.axon_stop_nrt_profile.restype = ctypes.c_int64

    @contextlib.contextmanager
    def hook(output_dir, device_ids):
        import jax

        jax.devices()
        if device_ids:
            ids = (ctypes.c_int64 * len(device_ids))(*device_ids)
            rc = lib.axon_start_nrt_profile(ids, len(device_ids))
        else:
            rc = lib.axon_start_nrt_profile(None, 0)
        if rc != 0:
            raise RuntimeError(f"axon_start_nrt_profile rc={rc}")
        try:
            yield
        finally:
            n = lib.axon_stop_nrt_profile(str(output_dir).encode())
            print(f"profile: {n} ntff file(s) in {output_dir}",
                  file=sys.stderr)

    mod = types.ModuleType("antenv.axon_hooks")
    mod.get_axon_ntff_profile_hook = lambda: hook
    mod.set_axon_ntff_profile_hook = lambda h: None
    sys.modules["antenv.axon_hooks"] = mod
    antenv.axon_hooks = mod


LAST_RESULTS = None


def kernel(_trace=False, **inputs):
    global _COMPILED, LAST_RESULTS
    from concourse import bass_utils

    if _trace:
        _install_trace_shim()

    if _COMPILED is None:
        _COMPILED = _build()
    nc = _COMPILED

    shared = _prep_shared_inputs(inputs)
    x = np.asarray(inputs["x"], dtype=np.float32)  # [B, T, D]
    in_maps = []
    for c in range(N_CORES):
        g, s = divmod(c, GROUP)
        xT_c = np.ascontiguousarray(x[g, s * S:(s + 1) * S, :].T.astype(_BF))
        m = dict(shared)
        m["xT"] = xT_c
        in_maps.append(m)

    LAST_RESULTS = bass_utils.run_bass_kernel_spmd(
        nc, in_maps, core_ids=list(range(N_CORES)), trace=_trace)

    out = np.empty((B, T, D), dtype=np.float32)
    for c in range(N_CORES):
        g, s = divmod(c, GROUP)
        out[g, s * S:(s + 1) * S, :] = LAST_RESULTS.results[c]["outT"].T
    return out
